# revision 21
# baseline (speedup 1.0000x reference)
"""MultiHeadCrossAttention TRN2 kernel (8 NeuronCores, SPMD).

Sharding: core c -> (batch b = c // 2, head-half hh = c % 2).
Head-half hh owns heads [hh, hh+2, ..., hh+14] (interleaved so both halves
get the same mix of projection-sourced and dec-sourced heads).

Key structural fact: the reference's "raw reshape" out.reshape(B, S, D)
maps head h's attention output rows [0..2048) x [0..64) onto rows
[h*128, (h+1)*128) of the pre-linear activation matrix. So each head's
full pipeline (qkv -> attention -> output linear rows) is independent;
no cross-core communication is needed.

Per head slot (8 per core), all layouts chosen so PE contractions are on
the partition dim:
  qkT  [128=64q+64k, S]  = W_qk block @ enc^T   (or dec^T slices direct)
  v    [S, 64] (+ones)   = enc @ Wv^T           (or dec slices direct)
  scoresT[k,q] tiles     = kT.T @ qT            (PSUM, fp32)
  probsT = exp(0.125*scoresT) * keepT           (ACT exp + DVE mask mult)
  attT_unnorm [65, S]    = [v|1].T @ probsT     (row 64 = softmax sums)
  attT = attT_unnorm * (1/sums) broadcast       (DVE recip + DMA bcast)
  out rows [128, 1024]   = sum_j attT_j.T @ linT_j + lin_b
"""

import numpy as np

B, S, D, H, HD = 4, 2048, 1024, 16, 64
NCORES = 8
NKT = D // 128  # 8 K-tiles over the enc feature dim

_CACHE = {}


def _heads_for(hh):
    return list(range(hh, H, 2))


def _build_nc(nslots=8, nphase=5):
    import concourse.bass as bass
    import concourse.tile as tile
    from concourse import bacc, mybir

    f32 = mybir.dt.float32
    bf16 = mybir.dt.bfloat16
    ts, ds = bass.ts, bass.ds

    nc = bacc.Bacc("TRN2", target_bir_lowering=False, debug=False,
                   num_devices=NCORES)

    # DRAM I/O (per-core contents differ; program is identical)
    xt_d = nc.dram_tensor("xt", [9, 128, S], bf16, kind="ExternalInput")
    qkdecT_d = nc.dram_tensor("qkdect", [4, 64, S], bf16, kind="ExternalInput")
    vdec_d = nc.dram_tensor("vdec", [128, 16, 3, 65], bf16, kind="ExternalInput")
    wqk_d = nc.dram_tensor("wqk", [128, 6, NKT, 128], bf16, kind="ExternalInput")
    qkb_d = nc.dram_tensor("qkb", [128, 6], f32, kind="ExternalInput")
    wv_d = nc.dram_tensor("wv", [128, 5, NKT, HD], bf16, kind="ExternalInput")
    vb_d = nc.dram_tensor("vb", [1, 5, 65], bf16, kind="ExternalInput")
    keep_d = nc.dram_tensor("keep", [16, 128, S], bf16, kind="ExternalInput")
    lin_d = nc.dram_tensor("lin", [128, 8, D], bf16, kind="ExternalInput")
    linb_d = nc.dram_tensor("linb", [1, D], bf16, kind="ExternalInput")
    out_d = nc.dram_tensor("out", [8, 128, D], f32, kind="ExternalOutput")

    with tile.TileContext(nc) as tc:
        with (
            tc.tile_pool(name="consts", bufs=1) as consts,
            tc.tile_pool(name="keepp", bufs=2) as keepp,
            tc.tile_pool(name="qkt", bufs=1) as qktp,
            tc.tile_pool(name="kt0", bufs=1) as kt0p,
            tc.tile_pool(name="vaug", bufs=2) as vaugp,
            tc.tile_pool(name="probs", bufs=16) as probsp,
            tc.tile_pool(name="attu", bufs=1) as attup,
            tc.tile_pool(name="recipb", bufs=1) as recipbp,
            tc.tile_pool(name="attn", bufs=2) as attnp,
            tc.tile_pool(name="rstage", bufs=1) as rstagep,
            tc.tile_pool(name="ps_small", bufs=2, space="PSUM") as ps_small,
            tc.tile_pool(name="ps_scores", bufs=2, space="PSUM") as ps_scores,
            tc.tile_pool(name="ps_att", bufs=2, space="PSUM") as ps_att,
        ):
            # ---- resident constants ----
            xt_sb = consts.tile([128, 9, S], bf16)
            for i in range(9):
                nc.sync.dma_start(out=xt_sb[:, i, :], in_=xt_d[i, :, :])
            qkdecT_sb = consts.tile([64, 4, S], bf16)
            for i in range(4):
                nc.sync.dma_start(out=qkdecT_sb[:, i, :], in_=qkdecT_d[i, :, :])
            vdec_sb = consts.tile([128, 16, 3, 65], bf16)
            nc.sync.dma_start(out=vdec_sb[:], in_=vdec_d[:, :, :, :])
            wqk_sb = consts.tile([128, 6, NKT, 128], bf16)
            nc.sync.dma_start(out=wqk_sb[:], in_=wqk_d[:, :, :, :])
            qkb_sb = consts.tile([128, 6], f32)
            nc.sync.dma_start(out=qkb_sb[:], in_=qkb_d[:, :])
            wv_sb = consts.tile([128, 5, NKT, HD], bf16)
            nc.sync.dma_start(out=wv_sb[:], in_=wv_d[:, :, :, :])
            vb_sb = consts.tile([1, 5, 65], bf16)
            nc.sync.dma_start(out=vb_sb[:], in_=vb_d[:, :, :])
            lin_sb = consts.tile([128, 8, D], bf16)
            nc.sync.dma_start(out=lin_sb[:], in_=lin_d[:, :, :])
            linb_sb = consts.tile([1, D], bf16)
            nc.sync.dma_start(out=linb_sb[:], in_=linb_d[:, :])
            ones_sb = consts.tile([1, 128], bf16)
            nc.vector.memset(ones_sb[:], 1.0)

            def xt_idx(slot, p):
                # slot 5 pass 7 reads the per-core extra tile (index 8)
                return p if not (slot == 5 and p == NKT - 1) else 8

            for slot in range(nslots):
                # ---------- projection ----------
                if slot < 6:
                    qkT = qktp.tile([128, S], bf16)
                    for c in range(4):
                        pq = ps_small.tile([128, 512], f32, tag="ps_small")
                        for p in range(NKT):
                            nc.tensor.matmul(
                                pq[:],
                                lhsT=wqk_sb[:, slot, p, :],
                                rhs=xt_sb[:, xt_idx(slot, p), ds(c * 512, 512)],
                                start=(p == 0),
                                stop=(p == NKT - 1),
                            )
                        nc.vector.tensor_scalar_add(
                            qkT[:, ds(c * 512, 512)], pq[:],
                            qkb_sb[:, slot:slot + 1],
                        )
                    # kT copy down to base partition 0 (matmul operands
                    # must share a base partition)
                    kT = kt0p.tile([64, S], bf16)
                    nc.sync.dma_start(out=kT[:], in_=qkT[64:128, :])
                    qT = qkT[0:64, :]
                    kT = kT[:]
                else:
                    qT = qkdecT_sb[:, 2 * (slot - 6), :]
                    kT = qkdecT_sb[:, 2 * (slot - 6) + 1, :]

                if slot < 5:
                    vaug = vaugp.tile([128, 16, 65], bf16)
                    for t in range(16):
                        pv = ps_small.tile([128, 65], f32, tag="ps_small")
                        nc.tensor.matmul(
                            pv[:, 0:65],
                            lhsT=ones_sb[0:1, :],
                            rhs=vb_sb[0:1, slot, :],
                            start=True, stop=False, skip_group_check=True,
                        )
                        for p in range(NKT):
                            nc.tensor.matmul(
                                pv[:, 0:HD],
                                lhsT=xt_sb[:, p, ts(t, 128)],
                                rhs=wv_sb[:, slot, p, :],
                                start=False, stop=(p == NKT - 1),
                                skip_group_check=True,
                            )
                        nc.vector.tensor_copy(vaug[:, t, :], pv[:])
                else:
                    vaug = vdec_sb[:, :, slot - 5, :]

                if nphase < 2:
                    continue
                # ---------- scoresT -> exp -> mask ----------
                probs = []
                for k in range(16):
                    pk = probsp.tile([128, S], bf16, tag="probs")
                    probs.append(pk)
                    keep_t = keepp.tile([128, S], bf16, tag="keepp")
                    nc.sync.dma_start(out=keep_t[:], in_=keep_d[k, :, :])
                    for half in range(2):
                        psc = ps_scores.tile([128, 1024], f32, tag="ps_scores")
                        for cc in range(2):
                            nc.tensor.matmul(
                                psc[:, ds(cc * 512, 512)],
                                lhsT=kT[:, ts(k, 128)],
                                rhs=qT[:, ds(half * 1024 + cc * 512, 512)],
                                start=True, stop=True,
                            )
                        import concourse.mybir as mybir_  # noqa
                        nc.scalar.activation(
                            out=pk[:, ds(half * 1024, 1024)],
                            in_=psc[:],
                            func=mybir_.ActivationFunctionType.Exp,
                            scale=0.125,
                        )
                        nc.vector.tensor_mul(
                            pk[:, ds(half * 1024, 1024)],
                            pk[:, ds(half * 1024, 1024)],
                            keep_t[:, ds(half * 1024, 1024)],
                        )

                if nphase < 3:
                    continue
                # ---------- attention x V (accumulate over k) ----------
                attu = attup.tile([66, S], f32, tag="attu")
                for c in range(4):
                    pa = ps_att.tile([65, 512], f32, tag="ps_att")
                    for k in range(16):
                        nc.tensor.matmul(
                            pa[:],
                            lhsT=vaug[:, k, :],
                            rhs=probs[k][:, ds(c * 512, 512)],
                            start=(k == 0), stop=(k == 15),
                        )
                    nc.vector.tensor_copy(attu[0:65, ds(c * 512, 512)], pa[:])

                if nphase < 4:
                    continue
                # ---------- normalize: 1/sums broadcast ----------
                rb = recipbp.tile([64, S], f32, tag="recipb")
                nc.sync.dma_start(out=rb[0:1, :], in_=attu[64:65, :])
                nc.vector.reciprocal(rb[0:1, :], rb[0:1, :])
                nc.gpsimd.partition_broadcast(rb[:], rb[0:1, :])

                attn_t = attnp.tile([128, 16, 128], bf16, tag="attn")
                nc.vector.tensor_mul(
                    attn_t[0:64, :, :],
                    attu[0:64, :].rearrange("p (r j) -> p j r", j=16),
                    rb[:].rearrange("p (r j) -> p j r", j=16),
                )
                # duplicate into partitions 64:128 so odd-j matmuls can use
                # base-64 operands on both sides
                nc.sync.dma_start(out=attn_t[64:128, :, :],
                                  in_=attn_t[0:64, :, :])

                if nphase < 5:
                    continue
                # ---------- output linear ----------
                # NB: matmuls with alternating operand base partitions inside
                # one accumulation group crash the runtime, so even j (base 0,
                # plus the bias pass) and odd j (base 64, using the duplicated
                # attn rows) accumulate in separate PSUM tiles, summed on DVE.
                rst = rstagep.tile([128, D], f32, tag="rstage")
                for n in range(2):
                    pre = ps_small.tile([128, 512], f32, tag="ps_small")
                    nc.tensor.matmul(
                        pre[:],
                        lhsT=ones_sb[0:1, :],
                        rhs=linb_sb[0:1, ds(n * 512, 512)],
                        start=True, stop=False, skip_group_check=True,
                    )
                    for j in range(0, 16, 2):
                        nc.tensor.matmul(
                            pre[:],
                            lhsT=attn_t[0:64, j, :],
                            rhs=lin_sb[0:64, j // 2, ds(n * 512, 512)],
                            start=False, stop=(j == 14),
                            skip_group_check=True,
                        )
                    pro = ps_small.tile([128, 512], f32, tag="ps_small")
                    for j in range(1, 16, 2):
                        nc.tensor.matmul(
                            pro[:],
                            lhsT=attn_t[64:128, j, :],
                            rhs=lin_sb[64:128, j // 2, ds(n * 512, 512)],
                            start=(j == 1), stop=(j == 15),
                            skip_group_check=True,
                        )
                    nc.vector.tensor_copy(rst[:, ds(n * 512, 512)], pre[:])
                    nc.vector.tensor_add(rst[:, ds(n * 512, 512)],
                                         rst[:, ds(n * 512, 512)], pro[:])
                nc.sync.dma_start(out=out_d[slot, :, :], in_=rst[:])

    nc.compile()
    return nc


def _prep_core_inputs(b, hh, dec_input, enc_input, keep16, W_qk_w, W_qk_b,
                      lin_in, lin_b16):
    import ml_dtypes
    bf16 = ml_dtypes.bfloat16
    heads = _heads_for(hh)
    enc_b = enc_input[b]
    dec_b = dec_input[b]
    encT = np.ascontiguousarray(enc_b.T)  # [1024, 2048]

    xt = np.empty((9, 128, S), np.float32)
    xt[:NKT] = encT.reshape(NKT, 128, S)
    if hh == 0:
        xt[8] = encT[896:1024]
    else:
        # head 11 q,k dec cols 64:192 transposed
        xt[8] = np.ascontiguousarray(dec_b[:, 64:192].T)

    qkdecT = np.empty((4, 64, S), np.float32)
    for i, slot in enumerate((6, 7)):
        h = heads[slot]
        mc = h * 192 - 2 * D  # dec col offset of this head's q
        qkdecT[2 * i] = dec_b[:, mc:mc + 64].T
        qkdecT[2 * i + 1] = dec_b[:, mc + 64:mc + 128].T

    vdec = np.empty((128, 16, 3, 65), np.float32)
    vdec[:, :, :, 64] = 1.0
    for blk, slot in enumerate((5, 6, 7)):
        h = heads[slot]
        mcv = h * 192 + 128 - 2 * D
        vcols = dec_b[:, mcv:mcv + 64]  # [2048, 64]
        vdec[:, :, blk, 0:64] = vcols.reshape(16, 128, 64).transpose(1, 0, 2)

    wqk = np.zeros((128, 6, NKT, 128), np.float32)
    qkb = np.zeros((128, 6), np.float32)
    for slot in range(6):
        h = heads[slot]
        if hh == 1 and slot == 5:
            wqk[:, slot, NKT - 1, :] = np.eye(128, dtype=np.float32)
        else:
            for p in range(NKT):
                wqk[:, slot, p, :] = W_qk_w[h * 192:h * 192 + 128,
                                            p * 128:(p + 1) * 128].T
            qkb[:, slot] = W_qk_b[h * 192:h * 192 + 128]

    wv = np.empty((128, 5, NKT, HD), np.float32)
    vb = np.empty((1, 5, 65), np.float32)
    vb[:, :, 64] = 1.0
    for slot in range(5):
        h = heads[slot]
        for p in range(NKT):
            wv[:, slot, p, :] = W_qk_w[h * 192 + 128:h * 192 + 192,
                                       p * 128:(p + 1) * 128].T
        vb[0, slot, 0:64] = W_qk_b[h * 192 + 128:h * 192 + 192]

    return {
        "xt": xt.astype(bf16),
        "qkdect": qkdecT.astype(bf16),
        "vdec": vdec.astype(bf16),
        "wqk": wqk.astype(bf16),
        "qkb": qkb,
        "wv": wv.astype(bf16),
        "vb": vb.astype(bf16),
        "keep": keep16,
        "lin": lin_in,
        "linb": lin_b16,
    }


def make_in_maps(dec_input, enc_input, mask, W_qk_w, W_qk_b, lin_w, lin_b):
    import ml_dtypes
    bf16 = ml_dtypes.bfloat16
    dec_input = np.asarray(dec_input, np.float32)
    enc_input = np.asarray(enc_input, np.float32)
    W_qk_w = np.asarray(W_qk_w, np.float32)
    W_qk_b = np.asarray(W_qk_b, np.float32)
    lin_w = np.asarray(lin_w, np.float32)
    lin_b = np.asarray(lin_b, np.float32)
    mask = np.asarray(mask)

    keep16 = np.ascontiguousarray(
        (~mask).T.astype(np.float32)).reshape(16, 128, S).astype(bf16)
    linT = np.ascontiguousarray(lin_w.T)  # [1024 (j,d), 1024 (n)]
    lin_in = np.ascontiguousarray(
        linT.reshape(8, 128, D).transpose(1, 0, 2)).astype(bf16)
    lin_b16 = lin_b.reshape(1, D).astype(bf16)

    in_maps = []
    for c in range(NCORES):
        b, hh = c // 2, c % 2
        in_maps.append(_prep_core_inputs(
            b, hh, dec_input, enc_input, keep16, W_qk_w, W_qk_b,
            lin_in, lin_b16))
    return in_maps


def get_nc():
    if "nc" not in _CACHE:
        _CACHE["nc"] = _build_nc()
    return _CACHE["nc"]


def gather_output(results):
    out = np.empty((B, S, D), np.float32)
    for c in range(NCORES):
        b, hh = c // 2, c % 2
        heads = _heads_for(hh)
        co = results[c]["out"]  # [8, 128, 1024]
        for slot, h in enumerate(heads):
            out[b, h * 128:(h + 1) * 128, :] = co[slot]
    return out


def kernel(dec_input, enc_input, mask, W_qk_w, W_qk_b, lin_w, lin_b):
    from concourse.bass_utils import run_bass_kernel_spmd

    nc = get_nc()
    in_maps = make_in_maps(dec_input, enc_input, mask, W_qk_w, W_qk_b,
                           lin_w, lin_b)
    res = run_bass_kernel_spmd(nc, in_maps, list(range(NCORES)))
    return gather_output(res.results)


# revision 22
# speedup vs baseline: 9.6213x; 9.6213x over previous
"""MultiHeadCrossAttention TRN2 kernel (8 NeuronCores, SPMD).

Sharding: core c -> (batch b = c // 2, head-half hh = c % 2).
Head-half hh owns heads [hh, hh+2, ..., hh+14] (interleaved so both halves
get the same mix of projection-sourced and dec-sourced heads).

Key structural fact: the reference's "raw reshape" out.reshape(B, S, D)
maps head h's attention output rows [0..2048) x [0..64) onto rows
[h*128, (h+1)*128) of the pre-linear activation matrix. So each head's
full pipeline (qkv -> attention -> output linear rows) is independent;
no cross-core communication is needed.

Per head slot (8 per core), all layouts chosen so PE contractions are on
the partition dim:
  qkT  [128=64q+64k, S]  = W_qk block @ enc^T   (or dec^T slices direct)
  v    [S, 64] (+ones)   = enc @ Wv^T           (or dec slices direct)
  scoresT[k,q] tiles     = kT.T @ qT            (PSUM, fp32)
  probsT = exp(0.125*scoresT) * keepT           (ACT exp + DVE mask mult)
  attT_unnorm [65, S]    = [v|1].T @ probsT     (row 64 = softmax sums)
  attT = attT_unnorm * (1/sums) broadcast       (DVE recip + DMA bcast)
  out rows [128, 1024]   = sum_j attT_j.T @ linT_j + lin_b
"""

import numpy as np

B, S, D, H, HD = 4, 2048, 1024, 16, 64
NCORES = 8
NKT = D // 128  # 8 K-tiles over the enc feature dim

_CACHE = {}


def _heads_for(hh):
    return list(range(hh, H, 2))


def _build_nc(nslots=8, nphase=5, nreps=1):
    import concourse.bass as bass
    import concourse.tile as tile
    from concourse import bacc, mybir

    f32 = mybir.dt.float32
    bf16 = mybir.dt.bfloat16
    ts, ds = bass.ts, bass.ds

    nc = bacc.Bacc("TRN2", target_bir_lowering=False, debug=False,
                   num_devices=NCORES)

    # DRAM I/O (per-core contents differ; program is identical)
    xt_d = nc.dram_tensor("xt", [9, 128, S], bf16, kind="ExternalInput")
    qkdecT_d = nc.dram_tensor("qkdect", [4, 64, S], bf16, kind="ExternalInput")
    vdec_d = nc.dram_tensor("vdec", [128, 16, 3, 65], bf16, kind="ExternalInput")
    wqk_d = nc.dram_tensor("wqk", [128, 6, NKT, 128], bf16, kind="ExternalInput")
    qkb_d = nc.dram_tensor("qkb", [128, 6], f32, kind="ExternalInput")
    wv_d = nc.dram_tensor("wv", [128, 5, NKT, HD], bf16, kind="ExternalInput")
    vb_d = nc.dram_tensor("vb", [1, 5, 65], bf16, kind="ExternalInput")
    keep_d = nc.dram_tensor("keep", [16, 128, S], bf16, kind="ExternalInput")
    lin_d = nc.dram_tensor("lin", [128, 8, D], bf16, kind="ExternalInput")
    linb_d = nc.dram_tensor("linb", [1, D], bf16, kind="ExternalInput")
    out_d = nc.dram_tensor("out", [8, 128, D], f32, kind="ExternalOutput")

    with tile.TileContext(nc) as tc:
        with (
            tc.tile_pool(name="consts", bufs=1) as consts,
            tc.tile_pool(name="keepp", bufs=2) as keepp,
            tc.tile_pool(name="qkt", bufs=1) as qktp,
            tc.tile_pool(name="kt0", bufs=1) as kt0p,
            tc.tile_pool(name="vaug", bufs=2) as vaugp,
            tc.tile_pool(name="probs", bufs=16) as probsp,
            tc.tile_pool(name="attu", bufs=1) as attup,
            tc.tile_pool(name="recipb", bufs=1) as recipbp,
            tc.tile_pool(name="attn", bufs=2) as attnp,
            tc.tile_pool(name="rstage", bufs=1) as rstagep,
            tc.tile_pool(name="ps_small", bufs=2, space="PSUM") as ps_small,
            tc.tile_pool(name="ps_scores", bufs=2, space="PSUM") as ps_scores,
            tc.tile_pool(name="ps_att", bufs=2, space="PSUM") as ps_att,
        ):
            # ---- resident constants ----
            xt_sb = consts.tile([128, 9, S], bf16)
            for i in range(9):
                nc.sync.dma_start(out=xt_sb[:, i, :], in_=xt_d[i, :, :])
            qkdecT_sb = consts.tile([64, 4, S], bf16)
            for i in range(4):
                nc.sync.dma_start(out=qkdecT_sb[:, i, :], in_=qkdecT_d[i, :, :])
            vdec_sb = consts.tile([128, 16, 3, 65], bf16)
            nc.sync.dma_start(out=vdec_sb[:], in_=vdec_d[:, :, :, :])
            wqk_sb = consts.tile([128, 6, NKT, 128], bf16)
            nc.sync.dma_start(out=wqk_sb[:], in_=wqk_d[:, :, :, :])
            qkb_sb = consts.tile([128, 6], f32)
            nc.sync.dma_start(out=qkb_sb[:], in_=qkb_d[:, :])
            wv_sb = consts.tile([128, 5, NKT, HD], bf16)
            nc.sync.dma_start(out=wv_sb[:], in_=wv_d[:, :, :, :])
            vb_sb = consts.tile([1, 5, 65], bf16)
            nc.sync.dma_start(out=vb_sb[:], in_=vb_d[:, :, :])
            lin_sb = consts.tile([128, 8, D], bf16)
            nc.sync.dma_start(out=lin_sb[:], in_=lin_d[:, :, :])
            linb_sb = consts.tile([1, D], bf16)
            nc.sync.dma_start(out=linb_sb[:], in_=linb_d[:, :])
            ones_sb = consts.tile([1, 128], bf16)
            nc.vector.memset(ones_sb[:], 1.0)

            def xt_idx(slot, p):
                # slot 5 pass 7 reads the per-core extra tile (index 8)
                return p if not (slot == 5 and p == NKT - 1) else 8

            for rep in range(nreps):
              for slot in range(nslots):
                # ---------- projection ----------
                if slot < 6:
                    qkT = qktp.tile([128, S], bf16)
                    for c in range(4):
                        pq = ps_small.tile([128, 512], f32, tag="ps_small")
                        for p in range(NKT):
                            nc.tensor.matmul(
                                pq[:],
                                lhsT=wqk_sb[:, slot, p, :],
                                rhs=xt_sb[:, xt_idx(slot, p), ds(c * 512, 512)],
                                start=(p == 0),
                                stop=(p == NKT - 1),
                            )
                        nc.vector.tensor_scalar_add(
                            qkT[:, ds(c * 512, 512)], pq[:],
                            qkb_sb[:, slot:slot + 1],
                        )
                    # kT copy down to base partition 0 (matmul operands
                    # must share a base partition)
                    kT = kt0p.tile([64, S], bf16)
                    nc.sync.dma_start(out=kT[:], in_=qkT[64:128, :])
                    qT = qkT[0:64, :]
                    kT = kT[:]
                else:
                    qT = qkdecT_sb[:, 2 * (slot - 6), :]
                    kT = qkdecT_sb[:, 2 * (slot - 6) + 1, :]

                if slot < 5:
                    vaug = vaugp.tile([128, 16, 65], bf16)
                    for t in range(16):
                        pv = ps_small.tile([128, 65], f32, tag="ps_small")
                        nc.tensor.matmul(
                            pv[:, 0:65],
                            lhsT=ones_sb[0:1, :],
                            rhs=vb_sb[0:1, slot, :],
                            start=True, stop=False, skip_group_check=True,
                        )
                        for p in range(NKT):
                            nc.tensor.matmul(
                                pv[:, 0:HD],
                                lhsT=xt_sb[:, p, ts(t, 128)],
                                rhs=wv_sb[:, slot, p, :],
                                start=False, stop=(p == NKT - 1),
                                skip_group_check=True,
                            )
                        nc.vector.tensor_copy(vaug[:, t, :], pv[:])
                else:
                    vaug = vdec_sb[:, :, slot - 5, :]

                if nphase < 2:
                    continue
                # ---------- scoresT -> exp -> mask ----------
                probs = []
                for k in range(16):
                    pk = probsp.tile([128, S], bf16, tag="probs")
                    probs.append(pk)
                    keep_t = keepp.tile([128, S], bf16, tag="keepp")
                    nc.sync.dma_start(out=keep_t[:], in_=keep_d[k, :, :])
                    for half in range(2):
                        psc = ps_scores.tile([128, 1024], f32, tag="ps_scores")
                        for cc in range(2):
                            nc.tensor.matmul(
                                psc[:, ds(cc * 512, 512)],
                                lhsT=kT[:, ts(k, 128)],
                                rhs=qT[:, ds(half * 1024 + cc * 512, 512)],
                                start=True, stop=True,
                            )
                        import concourse.mybir as mybir_  # noqa
                        nc.scalar.activation(
                            out=pk[:, ds(half * 1024, 1024)],
                            in_=psc[:],
                            func=mybir_.ActivationFunctionType.Exp,
                            scale=0.125,
                        )
                        nc.vector.tensor_mul(
                            pk[:, ds(half * 1024, 1024)],
                            pk[:, ds(half * 1024, 1024)],
                            keep_t[:, ds(half * 1024, 1024)],
                        )

                if nphase < 3:
                    continue
                # ---------- attention x V (accumulate over k) ----------
                attu = attup.tile([66, S], f32, tag="attu")
                for c in range(4):
                    pa = ps_att.tile([65, 512], f32, tag="ps_att")
                    for k in range(16):
                        nc.tensor.matmul(
                            pa[:],
                            lhsT=vaug[:, k, :],
                            rhs=probs[k][:, ds(c * 512, 512)],
                            start=(k == 0), stop=(k == 15),
                        )
                    nc.vector.tensor_copy(attu[0:65, ds(c * 512, 512)], pa[:])

                if nphase < 4:
                    continue
                # ---------- normalize: 1/sums broadcast ----------
                rb = recipbp.tile([64, S], f32, tag="recipb")
                nc.sync.dma_start(out=rb[0:1, :], in_=attu[64:65, :])
                nc.vector.reciprocal(rb[0:1, :], rb[0:1, :])
                nc.gpsimd.partition_broadcast(rb[:], rb[0:1, :])

                attn_t = attnp.tile([128, 16, 128], bf16, tag="attn")
                nc.vector.tensor_mul(
                    attn_t[0:64, :, :],
                    attu[0:64, :].rearrange("p (r j) -> p j r", j=16),
                    rb[:].rearrange("p (r j) -> p j r", j=16),
                )
                # duplicate into partitions 64:128 so odd-j matmuls can use
                # base-64 operands on both sides
                nc.sync.dma_start(out=attn_t[64:128, :, :],
                                  in_=attn_t[0:64, :, :])

                if nphase < 5:
                    continue
                # ---------- output linear ----------
                # NB: matmuls with alternating operand base partitions inside
                # one accumulation group crash the runtime, so even j (base 0,
                # plus the bias pass) and odd j (base 64, using the duplicated
                # attn rows) accumulate in separate PSUM tiles, summed on DVE.
                rst = rstagep.tile([128, D], f32, tag="rstage")
                for n in range(2):
                    pre = ps_small.tile([128, 512], f32, tag="ps_small")
                    nc.tensor.matmul(
                        pre[:],
                        lhsT=ones_sb[0:1, :],
                        rhs=linb_sb[0:1, ds(n * 512, 512)],
                        start=True, stop=False, skip_group_check=True,
                    )
                    for j in range(0, 16, 2):
                        nc.tensor.matmul(
                            pre[:],
                            lhsT=attn_t[0:64, j, :],
                            rhs=lin_sb[0:64, j // 2, ds(n * 512, 512)],
                            start=False, stop=(j == 14),
                            skip_group_check=True,
                        )
                    pro = ps_small.tile([128, 512], f32, tag="ps_small")
                    for j in range(1, 16, 2):
                        nc.tensor.matmul(
                            pro[:],
                            lhsT=attn_t[64:128, j, :],
                            rhs=lin_sb[64:128, j // 2, ds(n * 512, 512)],
                            start=(j == 1), stop=(j == 15),
                            skip_group_check=True,
                        )
                    nc.vector.tensor_copy(rst[:, ds(n * 512, 512)], pre[:])
                    nc.vector.tensor_add(rst[:, ds(n * 512, 512)],
                                         rst[:, ds(n * 512, 512)], pro[:])
                nc.sync.dma_start(out=out_d[slot, :, :], in_=rst[:])

    nc.compile()
    return nc


def _prep_core_inputs(b, hh, dec_input, enc_input, keep16, W_qk_w, W_qk_b,
                      lin_in, lin_b16):
    import ml_dtypes
    bf16 = ml_dtypes.bfloat16
    heads = _heads_for(hh)
    enc_b = enc_input[b]
    dec_b = dec_input[b]
    encT = np.ascontiguousarray(enc_b.T)  # [1024, 2048]

    xt = np.empty((9, 128, S), np.float32)
    xt[:NKT] = encT.reshape(NKT, 128, S)
    if hh == 0:
        xt[8] = encT[896:1024]
    else:
        # head 11 q,k dec cols 64:192 transposed
        xt[8] = np.ascontiguousarray(dec_b[:, 64:192].T)

    qkdecT = np.empty((4, 64, S), np.float32)
    for i, slot in enumerate((6, 7)):
        h = heads[slot]
        mc = h * 192 - 2 * D  # dec col offset of this head's q
        qkdecT[2 * i] = dec_b[:, mc:mc + 64].T
        qkdecT[2 * i + 1] = dec_b[:, mc + 64:mc + 128].T

    vdec = np.empty((128, 16, 3, 65), np.float32)
    vdec[:, :, :, 64] = 1.0
    for blk, slot in enumerate((5, 6, 7)):
        h = heads[slot]
        mcv = h * 192 + 128 - 2 * D
        vcols = dec_b[:, mcv:mcv + 64]  # [2048, 64]
        vdec[:, :, blk, 0:64] = vcols.reshape(16, 128, 64).transpose(1, 0, 2)

    wqk = np.zeros((128, 6, NKT, 128), np.float32)
    qkb = np.zeros((128, 6), np.float32)
    for slot in range(6):
        h = heads[slot]
        if hh == 1 and slot == 5:
            wqk[:, slot, NKT - 1, :] = np.eye(128, dtype=np.float32)
        else:
            for p in range(NKT):
                wqk[:, slot, p, :] = W_qk_w[h * 192:h * 192 + 128,
                                            p * 128:(p + 1) * 128].T
            qkb[:, slot] = W_qk_b[h * 192:h * 192 + 128]

    wv = np.empty((128, 5, NKT, HD), np.float32)
    vb = np.empty((1, 5, 65), np.float32)
    vb[:, :, 64] = 1.0
    for slot in range(5):
        h = heads[slot]
        for p in range(NKT):
            wv[:, slot, p, :] = W_qk_w[h * 192 + 128:h * 192 + 192,
                                       p * 128:(p + 1) * 128].T
        vb[0, slot, 0:64] = W_qk_b[h * 192 + 128:h * 192 + 192]

    return {
        "xt": xt.astype(bf16),
        "qkdect": qkdecT.astype(bf16),
        "vdec": vdec.astype(bf16),
        "wqk": wqk.astype(bf16),
        "qkb": qkb,
        "wv": wv.astype(bf16),
        "vb": vb.astype(bf16),
        "keep": keep16,
        "lin": lin_in,
        "linb": lin_b16,
    }


def make_in_maps(dec_input, enc_input, mask, W_qk_w, W_qk_b, lin_w, lin_b):
    import ml_dtypes
    bf16 = ml_dtypes.bfloat16
    dec_input = np.asarray(dec_input, np.float32)
    enc_input = np.asarray(enc_input, np.float32)
    W_qk_w = np.asarray(W_qk_w, np.float32)
    W_qk_b = np.asarray(W_qk_b, np.float32)
    lin_w = np.asarray(lin_w, np.float32)
    lin_b = np.asarray(lin_b, np.float32)
    mask = np.asarray(mask)

    keep16 = np.ascontiguousarray(
        (~mask).T.astype(np.float32)).reshape(16, 128, S).astype(bf16)
    linT = np.ascontiguousarray(lin_w.T)  # [1024 (j,d), 1024 (n)]
    lin_in = np.ascontiguousarray(
        linT.reshape(8, 128, D).transpose(1, 0, 2)).astype(bf16)
    lin_b16 = lin_b.reshape(1, D).astype(bf16)

    in_maps = []
    for c in range(NCORES):
        b, hh = c // 2, c % 2
        in_maps.append(_prep_core_inputs(
            b, hh, dec_input, enc_input, keep16, W_qk_w, W_qk_b,
            lin_in, lin_b16))
    return in_maps


def get_nc():
    if "nc" not in _CACHE:
        _CACHE["nc"] = _build_nc()
    return _CACHE["nc"]


def gather_output(results):
    out = np.empty((B, S, D), np.float32)
    for c in range(NCORES):
        b, hh = c // 2, c % 2
        heads = _heads_for(hh)
        co = results[c]["out"]  # [8, 128, 1024]
        for slot, h in enumerate(heads):
            out[b, h * 128:(h + 1) * 128, :] = co[slot]
    return out


def kernel(dec_input, enc_input, mask, W_qk_w, W_qk_b, lin_w, lin_b):
    from concourse.bass_utils import run_bass_kernel_spmd

    nc = get_nc()
    in_maps = make_in_maps(dec_input, enc_input, mask, W_qk_w, W_qk_b,
                           lin_w, lin_b)
    res = run_bass_kernel_spmd(nc, in_maps, list(range(NCORES)))
    return gather_output(res.results)


# revision 26
# speedup vs baseline: 9.7798x; 1.0165x over previous
"""MultiHeadCrossAttention TRN2 kernel (8 NeuronCores, SPMD).

Sharding: core c -> (batch b = c // 2, head-half hh = c % 2).
Head-half hh owns heads [hh, hh+2, ..., hh+14] (interleaved so both halves
get the same mix of projection-sourced and dec-sourced heads).

Key structural fact: the reference's "raw reshape" out.reshape(B, S, D)
maps head h's attention output rows [0..2048) x [0..64) onto rows
[h*128, (h+1)*128) of the pre-linear activation matrix. So each head's
full pipeline (qkv -> attention -> output linear rows) is independent;
no cross-core communication is needed.

Per head slot (8 per core), all layouts chosen so PE contractions are on
the partition dim:
  qkT  [128=64q+64k, S]  = W_qk block @ enc^T   (or dec^T slices direct)
  v    [S, 64] (+ones)   = enc @ Wv^T           (or dec slices direct)
  scoresT[k,q] tiles     = kT.T @ qT            (PSUM, fp32)
  probsT = exp(0.125*scoresT) * keepT           (ACT exp + DVE mask mult)
  attT_unnorm [65, S]    = [v|1].T @ probsT     (row 64 = softmax sums)
  attT = attT_unnorm * (1/sums) broadcast       (DVE recip + DMA bcast)
  out rows [128, 1024]   = sum_j attT_j.T @ linT_j + lin_b
"""

import numpy as np

B, S, D, H, HD = 4, 2048, 1024, 16, 64
NCORES = 8
NKT = D // 128  # 8 K-tiles over the enc feature dim

_CACHE = {}


def _heads_for(hh):
    return list(range(hh, H, 2))


def _build_nc(nslots=8, nphase=5, nreps=1):
    import concourse.bass as bass
    import concourse.tile as tile
    from concourse import bacc, mybir

    f32 = mybir.dt.float32
    bf16 = mybir.dt.bfloat16
    ts, ds = bass.ts, bass.ds

    nc = bacc.Bacc("TRN2", target_bir_lowering=False, debug=False,
                   num_devices=NCORES)

    # DRAM I/O (per-core contents differ; program is identical)
    xt_d = nc.dram_tensor("xt", [9, 128, S], bf16, kind="ExternalInput")
    qkdecT_d = nc.dram_tensor("qkdect", [4, 64, S], bf16, kind="ExternalInput")
    vdec_d = nc.dram_tensor("vdec", [128, 16, 3, 65], bf16, kind="ExternalInput")
    wqk_d = nc.dram_tensor("wqk", [128, 6, NKT, 128], bf16, kind="ExternalInput")
    qkb_d = nc.dram_tensor("qkb", [128, 6], f32, kind="ExternalInput")
    wv_d = nc.dram_tensor("wv", [128, 5, NKT, HD], bf16, kind="ExternalInput")
    vbp_d = nc.dram_tensor("vbp", [64, 5], f32, kind="ExternalInput")
    keep_d = nc.dram_tensor("keep", [16, 128, S], bf16, kind="ExternalInput")
    lin_d = nc.dram_tensor("lin", [128, 8, D], bf16, kind="ExternalInput")
    linb_d = nc.dram_tensor("linb", [1, D], bf16, kind="ExternalInput")
    out_d = nc.dram_tensor("out", [8, 128, D], f32, kind="ExternalOutput")

    with tile.TileContext(nc) as tc:
        with (
            tc.tile_pool(name="consts", bufs=1) as consts,
            tc.tile_pool(name="keepp", bufs=2) as keepp,
            tc.tile_pool(name="qkt", bufs=1) as qktp,
            tc.tile_pool(name="kt0", bufs=1) as kt0p,
            tc.tile_pool(name="vaug", bufs=2) as vaugp,
            tc.tile_pool(name="vt", bufs=1) as vtp,
            tc.tile_pool(name="probs", bufs=16) as probsp,
            tc.tile_pool(name="attu", bufs=1) as attup,
            tc.tile_pool(name="recipb", bufs=1) as recipbp,
            tc.tile_pool(name="attn", bufs=1) as attnp,
            tc.tile_pool(name="rstage", bufs=1) as rstagep,
            tc.tile_pool(name="ps_small", bufs=2, space="PSUM") as ps_small,
            tc.tile_pool(name="ps_scores", bufs=2, space="PSUM") as ps_scores,
            tc.tile_pool(name="ps_att", bufs=2, space="PSUM") as ps_att,
        ):
            # ---- resident constants ----
            xt_sb = consts.tile([128, 9, S], bf16)
            for i in range(9):
                nc.sync.dma_start(out=xt_sb[:, i, :], in_=xt_d[i, :, :])
            qkdecT_sb = consts.tile([64, 4, S], bf16)
            for i in range(4):
                nc.sync.dma_start(out=qkdecT_sb[:, i, :], in_=qkdecT_d[i, :, :])
            vdec_sb = consts.tile([128, 16, 3, 65], bf16)
            nc.sync.dma_start(out=vdec_sb[:], in_=vdec_d[:, :, :, :])
            wqk_sb = consts.tile([128, 6, NKT, 128], bf16)
            nc.sync.dma_start(out=wqk_sb[:], in_=wqk_d[:, :, :, :])
            qkb_sb = consts.tile([128, 6], f32)
            nc.sync.dma_start(out=qkb_sb[:], in_=qkb_d[:, :])
            wv_sb = consts.tile([128, 5, NKT, HD], bf16)
            nc.sync.dma_start(out=wv_sb[:], in_=wv_d[:, :, :, :])
            vbp_sb = consts.tile([64, 5], f32)
            nc.sync.dma_start(out=vbp_sb[:], in_=vbp_d[:, :])
            lin_sb = consts.tile([128, 8, D], bf16)
            nc.sync.dma_start(out=lin_sb[:], in_=lin_d[:, :, :])
            linb_sb = consts.tile([1, D], bf16)
            nc.sync.dma_start(out=linb_sb[:], in_=linb_d[:, :])
            ones_sb = consts.tile([1, 128], bf16)
            nc.vector.memset(ones_sb[:], 1.0)
            ident_sb = consts.tile([128, 128], bf16)
            from concourse.masks import make_identity
            make_identity(nc, ident_sb[:])

            def xt_idx(slot, p):
                # slot 5 pass 7 reads the per-core extra tile (index 8)
                return p if not (slot == 5 and p == NKT - 1) else 8

            for rep in range(nreps):
              for slot in range(nslots):
                # ---------- projection ----------
                if slot < 6:
                    qkT = qktp.tile([128, S], bf16)
                    for c in range(4):
                        pq = ps_small.tile([128, 512], f32, tag="ps_small")
                        for p in range(NKT):
                            nc.tensor.matmul(
                                pq[:],
                                lhsT=wqk_sb[:, slot, p, :],
                                rhs=xt_sb[:, xt_idx(slot, p), ds(c * 512, 512)],
                                start=(p == 0),
                                stop=(p == NKT - 1),
                            )
                        nc.vector.tensor_scalar_add(
                            qkT[:, ds(c * 512, 512)], pq[:],
                            qkb_sb[:, slot:slot + 1],
                        )
                    # kT copy down to base partition 0 (matmul operands
                    # must share a base partition)
                    kT = kt0p.tile([64, S], bf16)
                    nc.sync.dma_start(out=kT[:], in_=qkT[64:128, :])
                    qT = qkT[0:64, :]
                    kT = kT[:]
                else:
                    qT = qkdecT_sb[:, 2 * (slot - 6), :]
                    kT = qkdecT_sb[:, 2 * (slot - 6) + 1, :]

                if slot < 5:
                    # vT [65, S]: rows 0:64 = Wv @ enc^T (+bias), row 64 = 1
                    vT = vtp.tile([65, S], bf16, tag="vt")
                    nc.vector.memset(vT[64:65, :], 1.0)
                    for c in range(4):
                        pv = ps_small.tile([64, 512], f32, tag="ps_small")
                        for p in range(NKT):
                            nc.tensor.matmul(
                                pv[:],
                                lhsT=wv_sb[:, slot, p, :],
                                rhs=xt_sb[:, p, ds(c * 512, 512)],
                                start=(p == 0), stop=(p == NKT - 1),
                            )
                        nc.vector.tensor_scalar_add(
                            vT[0:64, ds(c * 512, 512)], pv[:],
                            vbp_sb[:, slot:slot + 1],
                        )
                    vaug = vaugp.tile([128, 16, 65], bf16)
                    for t in range(16):
                        pt = ps_small.tile([128, 65], bf16, tag="ps_small")
                        nc.tensor.transpose(pt[:], vT[:, ts(t, 128)],
                                            ident_sb[0:65, 0:65])
                        nc.vector.tensor_copy(vaug[:, t, :], pt[:])
                else:
                    vaug = vdec_sb[:, :, slot - 5, :]

                if nphase < 2:
                    continue
                # ---------- scoresT -> exp -> mask ----------
                probs = []
                for k in range(16):
                    pk = probsp.tile([128, S], bf16, tag="probs")
                    probs.append(pk)
                    keep_t = keepp.tile([128, S], bf16, tag="keepp")
                    nc.sync.dma_start(out=keep_t[:], in_=keep_d[k, :, :])
                    for half in range(2):
                        psc = ps_scores.tile([128, 1024], f32, tag="ps_scores")
                        for cc in range(2):
                            nc.tensor.matmul(
                                psc[:, ds(cc * 512, 512)],
                                lhsT=kT[:, ts(k, 128)],
                                rhs=qT[:, ds(half * 1024 + cc * 512, 512)],
                                start=True, stop=True,
                            )
                        import concourse.mybir as mybir_  # noqa
                        nc.scalar.activation(
                            out=pk[:, ds(half * 1024, 1024)],
                            in_=psc[:],
                            func=mybir_.ActivationFunctionType.Exp,
                            scale=0.125,
                        )
                        nc.vector.tensor_mul(
                            pk[:, ds(half * 1024, 1024)],
                            pk[:, ds(half * 1024, 1024)],
                            keep_t[:, ds(half * 1024, 1024)],
                        )

                if nphase < 3:
                    continue
                # ---------- attention x V (accumulate over k) ----------
                attu = attup.tile([66, S], f32, tag="attu")
                for c in range(4):
                    pa = ps_att.tile([65, 512], f32, tag="ps_att")
                    for k in range(16):
                        nc.tensor.matmul(
                            pa[:],
                            lhsT=vaug[:, k, :],
                            rhs=probs[k][:, ds(c * 512, 512)],
                            start=(k == 0), stop=(k == 15),
                        )
                    nc.vector.tensor_copy(attu[0:65, ds(c * 512, 512)], pa[:])

                if nphase < 4:
                    continue
                # ---------- normalize: 1/sums broadcast ----------
                rb = recipbp.tile([64, S], f32, tag="recipb")
                nc.sync.dma_start(out=rb[0:1, :], in_=attu[64:65, :])
                nc.vector.reciprocal(rb[0:1, :], rb[0:1, :])
                nc.gpsimd.partition_broadcast(rb[:], rb[0:1, :])

                attn_t = attnp.tile([128, 16, 128], bf16, tag="attn")
                nc.vector.tensor_mul(
                    attn_t[0:64, :, :],
                    attu[0:64, :].rearrange("p (r j) -> p j r", j=16),
                    rb[:].rearrange("p (r j) -> p j r", j=16),
                )
                # duplicate into partitions 64:128 so odd-j matmuls can use
                # base-64 operands on both sides
                nc.sync.dma_start(out=attn_t[64:128, :, :],
                                  in_=attn_t[0:64, :, :])

                if nphase < 5:
                    continue
                # ---------- output linear ----------
                # NB: matmuls with alternating operand base partitions inside
                # one accumulation group crash the runtime, so even j (base 0,
                # plus the bias pass) and odd j (base 64, using the duplicated
                # attn rows) accumulate in separate PSUM tiles, summed on DVE.
                rst = rstagep.tile([128, D], f32, tag="rstage")
                for n in range(2):
                    pre = ps_small.tile([128, 512], f32, tag="ps_small")
                    nc.tensor.matmul(
                        pre[:],
                        lhsT=ones_sb[0:1, :],
                        rhs=linb_sb[0:1, ds(n * 512, 512)],
                        start=True, stop=False, skip_group_check=True,
                    )
                    for j in range(0, 16, 2):
                        nc.tensor.matmul(
                            pre[:],
                            lhsT=attn_t[0:64, j, :],
                            rhs=lin_sb[0:64, j // 2, ds(n * 512, 512)],
                            start=False, stop=(j == 14),
                            skip_group_check=True,
                        )
                    pro = ps_small.tile([128, 512], f32, tag="ps_small")
                    for j in range(1, 16, 2):
                        nc.tensor.matmul(
                            pro[:],
                            lhsT=attn_t[64:128, j, :],
                            rhs=lin_sb[64:128, j // 2, ds(n * 512, 512)],
                            start=(j == 1), stop=(j == 15),
                            skip_group_check=True,
                        )
                    nc.vector.tensor_copy(rst[:, ds(n * 512, 512)], pre[:])
                    nc.vector.tensor_add(rst[:, ds(n * 512, 512)],
                                         rst[:, ds(n * 512, 512)], pro[:])
                nc.sync.dma_start(out=out_d[slot, :, :], in_=rst[:])

    nc.compile()
    return nc


def _prep_core_inputs(b, hh, dec_input, enc_input, keep16, W_qk_w, W_qk_b,
                      lin_in, lin_b16):
    import ml_dtypes
    bf16 = ml_dtypes.bfloat16
    heads = _heads_for(hh)
    enc_b = enc_input[b]
    dec_b = dec_input[b]
    encT = np.ascontiguousarray(enc_b.T)  # [1024, 2048]

    xt = np.empty((9, 128, S), np.float32)
    xt[:NKT] = encT.reshape(NKT, 128, S)
    if hh == 0:
        xt[8] = encT[896:1024]
    else:
        # head 11 q,k dec cols 64:192 transposed
        xt[8] = np.ascontiguousarray(dec_b[:, 64:192].T)

    qkdecT = np.empty((4, 64, S), np.float32)
    for i, slot in enumerate((6, 7)):
        h = heads[slot]
        mc = h * 192 - 2 * D  # dec col offset of this head's q
        qkdecT[2 * i] = dec_b[:, mc:mc + 64].T
        qkdecT[2 * i + 1] = dec_b[:, mc + 64:mc + 128].T

    vdec = np.empty((128, 16, 3, 65), np.float32)
    vdec[:, :, :, 64] = 1.0
    for blk, slot in enumerate((5, 6, 7)):
        h = heads[slot]
        mcv = h * 192 + 128 - 2 * D
        vcols = dec_b[:, mcv:mcv + 64]  # [2048, 64]
        vdec[:, :, blk, 0:64] = vcols.reshape(16, 128, 64).transpose(1, 0, 2)

    wqk = np.zeros((128, 6, NKT, 128), np.float32)
    qkb = np.zeros((128, 6), np.float32)
    for slot in range(6):
        h = heads[slot]
        if hh == 1 and slot == 5:
            wqk[:, slot, NKT - 1, :] = np.eye(128, dtype=np.float32)
        else:
            for p in range(NKT):
                wqk[:, slot, p, :] = W_qk_w[h * 192:h * 192 + 128,
                                            p * 128:(p + 1) * 128].T
            qkb[:, slot] = W_qk_b[h * 192:h * 192 + 128]

    wv = np.empty((128, 5, NKT, HD), np.float32)
    vbp = np.empty((64, 5), np.float32)
    for slot in range(5):
        h = heads[slot]
        for p in range(NKT):
            wv[:, slot, p, :] = W_qk_w[h * 192 + 128:h * 192 + 192,
                                       p * 128:(p + 1) * 128].T
        vbp[:, slot] = W_qk_b[h * 192 + 128:h * 192 + 192]

    return {
        "xt": xt.astype(bf16),
        "qkdect": qkdecT.astype(bf16),
        "vdec": vdec.astype(bf16),
        "wqk": wqk.astype(bf16),
        "qkb": qkb,
        "wv": wv.astype(bf16),
        "vbp": vbp,
        "keep": keep16,
        "lin": lin_in,
        "linb": lin_b16,
    }


def make_in_maps(dec_input, enc_input, mask, W_qk_w, W_qk_b, lin_w, lin_b):
    import ml_dtypes
    bf16 = ml_dtypes.bfloat16
    dec_input = np.asarray(dec_input, np.float32)
    enc_input = np.asarray(enc_input, np.float32)
    W_qk_w = np.asarray(W_qk_w, np.float32)
    W_qk_b = np.asarray(W_qk_b, np.float32)
    lin_w = np.asarray(lin_w, np.float32)
    lin_b = np.asarray(lin_b, np.float32)
    mask = np.asarray(mask)

    keep16 = np.ascontiguousarray(
        (~mask).T.astype(np.float32)).reshape(16, 128, S).astype(bf16)
    linT = np.ascontiguousarray(lin_w.T)  # [1024 (j,d), 1024 (n)]
    lin_in = np.ascontiguousarray(
        linT.reshape(8, 128, D).transpose(1, 0, 2)).astype(bf16)
    lin_b16 = lin_b.reshape(1, D).astype(bf16)

    in_maps = []
    for c in range(NCORES):
        b, hh = c // 2, c % 2
        in_maps.append(_prep_core_inputs(
            b, hh, dec_input, enc_input, keep16, W_qk_w, W_qk_b,
            lin_in, lin_b16))
    return in_maps


def get_nc():
    if "nc" not in _CACHE:
        _CACHE["nc"] = _build_nc()
    return _CACHE["nc"]


def gather_output(results):
    out = np.empty((B, S, D), np.float32)
    for c in range(NCORES):
        b, hh = c // 2, c % 2
        heads = _heads_for(hh)
        co = results[c]["out"]  # [8, 128, 1024]
        for slot, h in enumerate(heads):
            out[b, h * 128:(h + 1) * 128, :] = co[slot]
    return out


def kernel(dec_input, enc_input, mask, W_qk_w, W_qk_b, lin_w, lin_b):
    from concourse.bass_utils import run_bass_kernel_spmd

    nc = get_nc()
    in_maps = make_in_maps(dec_input, enc_input, mask, W_qk_w, W_qk_b,
                           lin_w, lin_b)
    res = run_bass_kernel_spmd(nc, in_maps, list(range(NCORES)))
    return gather_output(res.results)


# revision 27
# speedup vs baseline: 10.7477x; 1.0990x over previous
"""MultiHeadCrossAttention TRN2 kernel (8 NeuronCores, SPMD).

Sharding: core c -> (batch b = c // 2, head-half hh = c % 2).
Head-half hh owns heads [hh, hh+2, ..., hh+14] (interleaved so both halves
get the same mix of projection-sourced and dec-sourced heads).

Key structural fact: the reference's "raw reshape" out.reshape(B, S, D)
maps head h's attention output rows [0..2048) x [0..64) onto rows
[h*128, (h+1)*128) of the pre-linear activation matrix. So each head's
full pipeline (qkv -> attention -> output linear rows) is independent;
no cross-core communication is needed.

Per head slot (8 per core), all layouts chosen so PE contractions are on
the partition dim:
  qkT  [128=64q+64k, S]  = W_qk block @ enc^T   (or dec^T slices direct)
  v    [S, 64] (+ones)   = enc @ Wv^T           (or dec slices direct)
  scoresT[k,q] tiles     = kT.T @ qT            (PSUM, fp32)
  probsT = exp(0.125*scoresT) * keepT           (ACT exp + DVE mask mult)
  attT_unnorm [65, S]    = [v|1].T @ probsT     (row 64 = softmax sums)
  attT = attT_unnorm * (1/sums) broadcast       (DVE recip + DMA bcast)
  out rows [128, 1024]   = sum_j attT_j.T @ linT_j + lin_b
"""

import numpy as np

B, S, D, H, HD = 4, 2048, 1024, 16, 64
NCORES = 8
NKT = D // 128  # 8 K-tiles over the enc feature dim

_CACHE = {}


def _heads_for(hh):
    return list(range(hh, H, 2))


def _build_nc(nslots=8, nphase=5, nreps=1):
    import concourse.bass as bass
    import concourse.tile as tile
    from concourse import bacc, mybir

    f32 = mybir.dt.float32
    bf16 = mybir.dt.bfloat16
    ts, ds = bass.ts, bass.ds

    nc = bacc.Bacc("TRN2", target_bir_lowering=False, debug=False,
                   num_devices=NCORES)

    # DRAM I/O (per-core contents differ; program is identical)
    xt_d = nc.dram_tensor("xt", [9, 128, S], bf16, kind="ExternalInput")
    qkdecT_d = nc.dram_tensor("qkdect", [4, 64, S], bf16, kind="ExternalInput")
    vdec_d = nc.dram_tensor("vdec", [128, 16, 3, 65], bf16, kind="ExternalInput")
    wqk_d = nc.dram_tensor("wqk", [128, 6, NKT, 128], bf16, kind="ExternalInput")
    qkb_d = nc.dram_tensor("qkb", [128, 6], f32, kind="ExternalInput")
    wv_d = nc.dram_tensor("wv", [128, 5, NKT, HD], bf16, kind="ExternalInput")
    vbp_d = nc.dram_tensor("vbp", [64, 5], f32, kind="ExternalInput")
    keep_d = nc.dram_tensor("keep", [16, 128, S], bf16, kind="ExternalInput")
    lin_d = nc.dram_tensor("lin", [128, 8, D], bf16, kind="ExternalInput")
    linb_d = nc.dram_tensor("linb", [1, D], bf16, kind="ExternalInput")
    out_d = nc.dram_tensor("out", [8, 128, D], f32, kind="ExternalOutput")

    with tile.TileContext(nc) as tc:
        with (
            tc.tile_pool(name="consts", bufs=1) as consts,
            tc.tile_pool(name="keepp", bufs=2) as keepp,
            tc.tile_pool(name="qkt", bufs=1) as qktp,
            tc.tile_pool(name="kt0", bufs=1) as kt0p,
            tc.tile_pool(name="vaug", bufs=2) as vaugp,
            tc.tile_pool(name="vt", bufs=1) as vtp,
            tc.tile_pool(name="probs", bufs=16) as probsp,
            tc.tile_pool(name="attu", bufs=1) as attup,
            tc.tile_pool(name="recipb", bufs=1) as recipbp,
            tc.tile_pool(name="attn", bufs=1) as attnp,
            tc.tile_pool(name="odd", bufs=1) as oddp,
            tc.tile_pool(name="rstage", bufs=1) as rstagep,
            tc.tile_pool(name="ps_small", bufs=2, space="PSUM") as ps_small,
            tc.tile_pool(name="ps_scores", bufs=2, space="PSUM") as ps_scores,
            tc.tile_pool(name="ps_att", bufs=2, space="PSUM") as ps_att,
        ):
            # ---- resident constants ----
            xt_sb = consts.tile([128, 9, S], bf16)
            for i in range(9):
                nc.sync.dma_start(out=xt_sb[:, i, :], in_=xt_d[i, :, :])
            qkdecT_sb = consts.tile([64, 4, S], bf16)
            for i in range(4):
                nc.sync.dma_start(out=qkdecT_sb[:, i, :], in_=qkdecT_d[i, :, :])
            vdec_sb = consts.tile([128, 16, 3, 65], bf16)
            nc.sync.dma_start(out=vdec_sb[:], in_=vdec_d[:, :, :, :])
            wqk_sb = consts.tile([128, 6, NKT, 128], bf16)
            nc.sync.dma_start(out=wqk_sb[:], in_=wqk_d[:, :, :, :])
            qkb_sb = consts.tile([128, 6], f32)
            nc.sync.dma_start(out=qkb_sb[:], in_=qkb_d[:, :])
            wv_sb = consts.tile([128, 5, NKT, HD], bf16)
            nc.sync.dma_start(out=wv_sb[:], in_=wv_d[:, :, :, :])
            vbp_sb = consts.tile([64, 5], f32)
            nc.sync.dma_start(out=vbp_sb[:], in_=vbp_d[:, :])
            lin_sb = consts.tile([128, 8, D], bf16)
            nc.sync.dma_start(out=lin_sb[:], in_=lin_d[:, :, :])
            linb_sb = consts.tile([1, D], bf16)
            nc.sync.dma_start(out=linb_sb[:], in_=linb_d[:, :])
            ones_sb = consts.tile([1, 128], bf16)
            nc.vector.memset(ones_sb[:], 1.0)
            ident_sb = consts.tile([128, 128], bf16)
            from concourse.masks import make_identity
            make_identity(nc, ident_sb[:])

            def xt_idx(slot, p):
                # slot 5 pass 7 reads the per-core extra tile (index 8)
                return p if not (slot == 5 and p == NKT - 1) else 8

            for rep in range(nreps):
              for slot in range(nslots):
                # ---------- projection ----------
                if slot < 6:
                    qkT = qktp.tile([128, S], bf16)
                    for c in range(4):
                        pq = ps_small.tile([128, 512], f32, tag="ps_small")
                        for p in range(NKT):
                            nc.tensor.matmul(
                                pq[:],
                                lhsT=wqk_sb[:, slot, p, :],
                                rhs=xt_sb[:, xt_idx(slot, p), ds(c * 512, 512)],
                                start=(p == 0),
                                stop=(p == NKT - 1),
                            )
                        nc.vector.tensor_scalar_add(
                            qkT[:, ds(c * 512, 512)], pq[:],
                            qkb_sb[:, slot:slot + 1],
                        )
                    # kT copy down to base partition 0 (matmul operands
                    # must share a base partition)
                    kT = kt0p.tile([64, S], bf16)
                    nc.sync.dma_start(out=kT[:], in_=qkT[64:128, :])
                    qT = qkT[0:64, :]
                    kT = kT[:]
                else:
                    qT = qkdecT_sb[:, 2 * (slot - 6), :]
                    kT = qkdecT_sb[:, 2 * (slot - 6) + 1, :]

                if slot < 5:
                    # vT [65, S]: rows 0:64 = Wv @ enc^T (+bias), row 64 = 1
                    vT = vtp.tile([65, S], bf16, tag="vt")
                    nc.vector.memset(vT[64:65, :], 1.0)
                    for c in range(4):
                        pv = ps_small.tile([64, 512], f32, tag="ps_small")
                        for p in range(NKT):
                            nc.tensor.matmul(
                                pv[:],
                                lhsT=wv_sb[:, slot, p, :],
                                rhs=xt_sb[:, p, ds(c * 512, 512)],
                                start=(p == 0), stop=(p == NKT - 1),
                            )
                        nc.vector.tensor_scalar_add(
                            vT[0:64, ds(c * 512, 512)], pv[:],
                            vbp_sb[:, slot:slot + 1],
                        )
                    vaug = vaugp.tile([128, 16, 65], bf16)
                    for t in range(16):
                        pt = ps_small.tile([128, 65], bf16, tag="ps_small")
                        nc.tensor.transpose(pt[:], vT[:, ts(t, 128)],
                                            ident_sb[0:65, 0:65])
                        nc.vector.tensor_copy(vaug[:, t, :], pt[:])
                else:
                    vaug = vdec_sb[:, :, slot - 5, :]

                if nphase < 2:
                    continue
                # ---------- scoresT -> exp -> mask ----------
                probs = []
                for k in range(16):
                    pk = probsp.tile([128, S], bf16, tag="probs")
                    probs.append(pk)
                    keep_t = keepp.tile([128, S], bf16, tag="keepp")
                    nc.sync.dma_start(out=keep_t[:], in_=keep_d[k, :, :])
                    for half in range(2):
                        psc = ps_scores.tile([128, 1024], f32, tag="ps_scores")
                        for cc in range(2):
                            nc.tensor.matmul(
                                psc[:, ds(cc * 512, 512)],
                                lhsT=kT[:, ts(k, 128)],
                                rhs=qT[:, ds(half * 1024 + cc * 512, 512)],
                                start=True, stop=True,
                            )
                        import concourse.mybir as mybir_  # noqa
                        nc.scalar.activation(
                            out=pk[:, ds(half * 1024, 1024)],
                            in_=psc[:],
                            func=mybir_.ActivationFunctionType.Exp,
                            scale=0.125,
                        )
                        nc.vector.tensor_mul(
                            pk[:, ds(half * 1024, 1024)],
                            pk[:, ds(half * 1024, 1024)],
                            keep_t[:, ds(half * 1024, 1024)],
                        )

                if nphase < 3:
                    continue
                # ---------- attention x V (accumulate over k) ----------
                attu = attup.tile([66, S], f32, tag="attu")
                for c in range(4):
                    pa = ps_att.tile([65, 512], f32, tag="ps_att")
                    for k in range(16):
                        nc.tensor.matmul(
                            pa[:],
                            lhsT=vaug[:, k, :],
                            rhs=probs[k][:, ds(c * 512, 512)],
                            start=(k == 0), stop=(k == 15),
                        )
                    nc.vector.tensor_copy(attu[0:65, ds(c * 512, 512)], pa[:])

                if nphase < 4:
                    continue
                # ---------- normalize: 1/sums broadcast ----------
                rb = recipbp.tile([64, S], f32, tag="recipb")
                nc.sync.dma_start(out=rb[0:1, :], in_=attu[64:65, :])
                nc.vector.reciprocal(rb[0:1, :], rb[0:1, :])
                nc.gpsimd.partition_broadcast(rb[:], rb[0:1, :])

                # attn_t pairs j=(2jj, 2jj+1) across partition halves so the
                # output linear contracts K=128 per pair-matmul. DVE lanes can
                # only write partitions 0:64, so odd-j blocks stage through
                # odd_t and DMA up to partitions 64:128.
                attn_t = attnp.tile([128, 8, 128], bf16, tag="attn")
                odd_t = oddp.tile([64, 8, 128], bf16, tag="odd")
                a_r = attu[0:64, :].rearrange("p (r jj e) -> p e jj r",
                                              jj=8, e=2)
                b_r = rb[:].rearrange("p (r jj e) -> p e jj r", jj=8, e=2)
                nc.vector.tensor_mul(attn_t[0:64, :, :],
                                     a_r[:, 0, :, :], b_r[:, 0, :, :])
                nc.vector.tensor_mul(odd_t[:], a_r[:, 1, :, :],
                                     b_r[:, 1, :, :])
                nc.sync.dma_start(out=attn_t[64:128, :, :], in_=odd_t[:])

                if nphase < 5:
                    continue
                # ---------- output linear ----------
                rst = rstagep.tile([128, D], f32, tag="rstage")
                for n in range(2):
                    pr = ps_small.tile([128, 512], f32, tag="ps_small")
                    nc.tensor.matmul(
                        pr[:],
                        lhsT=ones_sb[0:1, :],
                        rhs=linb_sb[0:1, ds(n * 512, 512)],
                        start=True, stop=False, skip_group_check=True,
                    )
                    for jj in range(8):
                        nc.tensor.matmul(
                            pr[:],
                            lhsT=attn_t[:, jj, :],
                            rhs=lin_sb[:, jj, ds(n * 512, 512)],
                            start=False, stop=(jj == 7),
                            skip_group_check=True,
                        )
                    nc.vector.tensor_copy(rst[:, ds(n * 512, 512)], pr[:])
                nc.sync.dma_start(out=out_d[slot, :, :], in_=rst[:])

    nc.compile()
    return nc


def _prep_core_inputs(b, hh, dec_input, enc_input, keep16, W_qk_w, W_qk_b,
                      lin_in, lin_b16):
    import ml_dtypes
    bf16 = ml_dtypes.bfloat16
    heads = _heads_for(hh)
    enc_b = enc_input[b]
    dec_b = dec_input[b]
    encT = np.ascontiguousarray(enc_b.T)  # [1024, 2048]

    xt = np.empty((9, 128, S), np.float32)
    xt[:NKT] = encT.reshape(NKT, 128, S)
    if hh == 0:
        xt[8] = encT[896:1024]
    else:
        # head 11 q,k dec cols 64:192 transposed
        xt[8] = np.ascontiguousarray(dec_b[:, 64:192].T)

    qkdecT = np.empty((4, 64, S), np.float32)
    for i, slot in enumerate((6, 7)):
        h = heads[slot]
        mc = h * 192 - 2 * D  # dec col offset of this head's q
        qkdecT[2 * i] = dec_b[:, mc:mc + 64].T
        qkdecT[2 * i + 1] = dec_b[:, mc + 64:mc + 128].T

    vdec = np.empty((128, 16, 3, 65), np.float32)
    vdec[:, :, :, 64] = 1.0
    for blk, slot in enumerate((5, 6, 7)):
        h = heads[slot]
        mcv = h * 192 + 128 - 2 * D
        vcols = dec_b[:, mcv:mcv + 64]  # [2048, 64]
        vdec[:, :, blk, 0:64] = vcols.reshape(16, 128, 64).transpose(1, 0, 2)

    wqk = np.zeros((128, 6, NKT, 128), np.float32)
    qkb = np.zeros((128, 6), np.float32)
    for slot in range(6):
        h = heads[slot]
        if hh == 1 and slot == 5:
            wqk[:, slot, NKT - 1, :] = np.eye(128, dtype=np.float32)
        else:
            for p in range(NKT):
                wqk[:, slot, p, :] = W_qk_w[h * 192:h * 192 + 128,
                                            p * 128:(p + 1) * 128].T
            qkb[:, slot] = W_qk_b[h * 192:h * 192 + 128]

    wv = np.empty((128, 5, NKT, HD), np.float32)
    vbp = np.empty((64, 5), np.float32)
    for slot in range(5):
        h = heads[slot]
        for p in range(NKT):
            wv[:, slot, p, :] = W_qk_w[h * 192 + 128:h * 192 + 192,
                                       p * 128:(p + 1) * 128].T
        vbp[:, slot] = W_qk_b[h * 192 + 128:h * 192 + 192]

    return {
        "xt": xt.astype(bf16),
        "qkdect": qkdecT.astype(bf16),
        "vdec": vdec.astype(bf16),
        "wqk": wqk.astype(bf16),
        "qkb": qkb,
        "wv": wv.astype(bf16),
        "vbp": vbp,
        "keep": keep16,
        "lin": lin_in,
        "linb": lin_b16,
    }


def make_in_maps(dec_input, enc_input, mask, W_qk_w, W_qk_b, lin_w, lin_b):
    import ml_dtypes
    bf16 = ml_dtypes.bfloat16
    dec_input = np.asarray(dec_input, np.float32)
    enc_input = np.asarray(enc_input, np.float32)
    W_qk_w = np.asarray(W_qk_w, np.float32)
    W_qk_b = np.asarray(W_qk_b, np.float32)
    lin_w = np.asarray(lin_w, np.float32)
    lin_b = np.asarray(lin_b, np.float32)
    mask = np.asarray(mask)

    keep16 = np.ascontiguousarray(
        (~mask).T.astype(np.float32)).reshape(16, 128, S).astype(bf16)
    linT = np.ascontiguousarray(lin_w.T)  # [1024 (j,d), 1024 (n)]
    lin_in = np.ascontiguousarray(
        linT.reshape(8, 128, D).transpose(1, 0, 2)).astype(bf16)
    lin_b16 = lin_b.reshape(1, D).astype(bf16)

    in_maps = []
    for c in range(NCORES):
        b, hh = c // 2, c % 2
        in_maps.append(_prep_core_inputs(
            b, hh, dec_input, enc_input, keep16, W_qk_w, W_qk_b,
            lin_in, lin_b16))
    return in_maps


def get_nc():
    if "nc" not in _CACHE:
        _CACHE["nc"] = _build_nc()
    return _CACHE["nc"]


def gather_output(results):
    out = np.empty((B, S, D), np.float32)
    for c in range(NCORES):
        b, hh = c // 2, c % 2
        heads = _heads_for(hh)
        co = results[c]["out"]  # [8, 128, 1024]
        for slot, h in enumerate(heads):
            out[b, h * 128:(h + 1) * 128, :] = co[slot]
    return out


def kernel(dec_input, enc_input, mask, W_qk_w, W_qk_b, lin_w, lin_b):
    from concourse.bass_utils import run_bass_kernel_spmd

    nc = get_nc()
    in_maps = make_in_maps(dec_input, enc_input, mask, W_qk_w, W_qk_b,
                           lin_w, lin_b)
    res = run_bass_kernel_spmd(nc, in_maps, list(range(NCORES)))
    return gather_output(res.results)


# revision 29
# speedup vs baseline: 11.6653x; 1.0854x over previous
"""MultiHeadCrossAttention TRN2 kernel (8 NeuronCores, SPMD).

Sharding: core c -> (batch b = c // 2, head-half hh = c % 2).
Head-half hh owns heads [hh, hh+2, ..., hh+14] (interleaved so both halves
get the same mix of projection-sourced and dec-sourced heads).

Key structural fact: the reference's "raw reshape" out.reshape(B, S, D)
maps head h's attention output rows [0..2048) x [0..64) onto rows
[h*128, (h+1)*128) of the pre-linear activation matrix. So each head's
full pipeline (qkv -> attention -> output linear rows) is independent;
no cross-core communication is needed.

Per head slot (8 per core), all layouts chosen so PE contractions are on
the partition dim:
  qkT  [128=64q+64k, S]  = W_qk block @ enc^T   (or dec^T slices direct)
  v    [S, 64] (+ones)   = enc @ Wv^T           (or dec slices direct)
  scoresT[k,q] tiles     = kT.T @ qT            (PSUM, fp32)
  probsT = exp(0.125*scoresT) * keepT           (ACT exp + DVE mask mult)
  attT_unnorm [65, S]    = [v|1].T @ probsT     (row 64 = softmax sums)
  attT = attT_unnorm * (1/sums) broadcast       (DVE recip + DMA bcast)
  out rows [128, 1024]   = sum_j attT_j.T @ linT_j + lin_b
"""

import numpy as np

B, S, D, H, HD = 4, 2048, 1024, 16, 64
NCORES = 8
NKT = D // 128  # 8 K-tiles over the enc feature dim

_CACHE = {}


def _heads_for(hh):
    return list(range(hh, H, 2))


def _build_nc(nslots=8, nphase=5, nreps=1):
    import concourse.bass as bass
    import concourse.tile as tile
    from concourse import bacc, mybir

    f32 = mybir.dt.float32
    bf16 = mybir.dt.bfloat16
    ts, ds = bass.ts, bass.ds

    nc = bacc.Bacc("TRN2", target_bir_lowering=False, debug=False,
                   num_devices=NCORES)

    # DRAM I/O (per-core contents differ; program is identical)
    xt_d = nc.dram_tensor("xt", [9, 128, S], bf16, kind="ExternalInput")
    qkdecT_d = nc.dram_tensor("qkdect", [4, 64, S], bf16, kind="ExternalInput")
    vdec_d = nc.dram_tensor("vdec", [128, 16, 3, 65], bf16, kind="ExternalInput")
    wqk_d = nc.dram_tensor("wqk", [128, 6, NKT, 128], bf16, kind="ExternalInput")
    qkb_d = nc.dram_tensor("qkb", [128, 6], f32, kind="ExternalInput")
    wv_d = nc.dram_tensor("wv", [128, 5, NKT, HD], bf16, kind="ExternalInput")
    vbp_d = nc.dram_tensor("vbp", [64, 5], f32, kind="ExternalInput")
    keep_d = nc.dram_tensor("keep", [16, 128, S], bf16, kind="ExternalInput")
    lin_d = nc.dram_tensor("lin", [128, 8, D], bf16, kind="ExternalInput")
    linb_d = nc.dram_tensor("linb", [1, D], bf16, kind="ExternalInput")
    out_d = nc.dram_tensor("out", [8, 128, D], f32, kind="ExternalOutput")

    with tile.TileContext(nc) as tc:
        with (
            tc.tile_pool(name="consts", bufs=1) as consts,
            tc.tile_pool(name="keepp", bufs=2) as keepp,
            tc.tile_pool(name="qkt", bufs=1) as qktp,
            tc.tile_pool(name="kt0", bufs=1) as kt0p,
            tc.tile_pool(name="vaug", bufs=2) as vaugp,
            tc.tile_pool(name="vt", bufs=1) as vtp,
            tc.tile_pool(name="probs", bufs=16) as probsp,
            tc.tile_pool(name="attu", bufs=1) as attup,
            tc.tile_pool(name="recipb", bufs=1) as recipbp,
            tc.tile_pool(name="attn", bufs=1) as attnp,
            tc.tile_pool(name="odd", bufs=1) as oddp,
            tc.tile_pool(name="rstage", bufs=1) as rstagep,
            tc.tile_pool(name="ps_small", bufs=2, space="PSUM") as ps_small,
            tc.tile_pool(name="ps_scores", bufs=2, space="PSUM") as ps_scores,
            tc.tile_pool(name="ps_att", bufs=1, space="PSUM") as ps_att,
        ):
            # ---- resident constants ----
            xt_sb = consts.tile([128, 9, S], bf16)
            for i in range(9):
                nc.sync.dma_start(out=xt_sb[:, i, :], in_=xt_d[i, :, :])
            wqk_sb = consts.tile([128, 6, NKT, 128], bf16)
            nc.sync.dma_start(out=wqk_sb[:], in_=wqk_d[:, :, :, :])
            qkb_sb = consts.tile([128, 6], f32)
            nc.sync.dma_start(out=qkb_sb[:], in_=qkb_d[:, :])
            wv_sb = consts.tile([128, 5, NKT, HD], bf16)
            nc.sync.dma_start(out=wv_sb[:], in_=wv_d[:, :, :, :])
            qkdecT_sb = consts.tile([64, 4, S], bf16)
            for i in range(4):
                nc.sync.dma_start(out=qkdecT_sb[:, i, :], in_=qkdecT_d[i, :, :])
            vdec_sb = consts.tile([128, 16, 3, 65], bf16)
            nc.sync.dma_start(out=vdec_sb[:], in_=vdec_d[:, :, :, :])
            vbp_sb = consts.tile([64, 5], f32)
            nc.sync.dma_start(out=vbp_sb[:], in_=vbp_d[:, :])
            lin_sb = consts.tile([128, 8, D], bf16)
            nc.sync.dma_start(out=lin_sb[:], in_=lin_d[:, :, :])
            linb_sb = consts.tile([1, D], bf16)
            nc.sync.dma_start(out=linb_sb[:], in_=linb_d[:, :])
            ones_sb = consts.tile([1, 128], bf16)
            nc.vector.memset(ones_sb[:], 1.0)
            ident_sb = consts.tile([128, 128], bf16)
            from concourse.masks import make_identity
            make_identity(nc, ident_sb[:])

            def xt_idx(slot, p):
                # slot 5 pass 7 reads the per-core extra tile (index 8)
                return p if not (slot == 5 and p == NKT - 1) else 8

            def emit_final(slot, attn_t):
                # ---------- output linear (for the previous slot) ----------
                rst = rstagep.tile([128, D], f32, tag="rstage")
                for n in range(2):
                    pr = ps_small.tile([128, 512], f32, tag="ps_small")
                    nc.tensor.matmul(
                        pr[:],
                        lhsT=ones_sb[0:1, :],
                        rhs=linb_sb[0:1, ds(n * 512, 512)],
                        start=True, stop=False, skip_group_check=True,
                    )
                    for jj in range(8):
                        nc.tensor.matmul(
                            pr[:],
                            lhsT=attn_t[:, jj, :],
                            rhs=lin_sb[:, jj, ds(n * 512, 512)],
                            start=False, stop=(jj == 7),
                            skip_group_check=True,
                        )
                    nc.vector.tensor_copy(rst[:, ds(n * 512, 512)], pr[:])
                nc.sync.dma_start(out=out_d[slot, :, :], in_=rst[:])

            pending = None  # (slot, attn_t) awaiting its output linear
            for rep in range(nreps):
              for slot in range(nslots):
                # ---------- projection ----------
                if slot < 6:
                    qkT = qktp.tile([128, S], bf16)
                    for c in range(4):
                        pq = ps_small.tile([128, 512], f32, tag="ps_small")
                        for p in range(NKT):
                            nc.tensor.matmul(
                                pq[:],
                                lhsT=wqk_sb[:, slot, p, :],
                                rhs=xt_sb[:, xt_idx(slot, p), ds(c * 512, 512)],
                                start=(p == 0),
                                stop=(p == NKT - 1),
                            )
                        nc.vector.tensor_scalar_add(
                            qkT[:, ds(c * 512, 512)], pq[:],
                            qkb_sb[:, slot:slot + 1],
                        )
                    # kT copy down to base partition 0 (matmul operands
                    # must share a base partition)
                    kT = kt0p.tile([64, S], bf16)
                    nc.sync.dma_start(out=kT[:], in_=qkT[64:128, :])
                    qT = qkT[0:64, :]
                    kT = kT[:]
                else:
                    qT = qkdecT_sb[:, 2 * (slot - 6), :]
                    kT = qkdecT_sb[:, 2 * (slot - 6) + 1, :]

                if slot < 5:
                    # vT [65, S]: rows 0:64 = Wv @ enc^T (+bias), row 64 = 1
                    vT = vtp.tile([65, S], bf16, tag="vt")
                    nc.vector.memset(vT[64:65, :], 1.0)
                    for c in range(4):
                        pv = ps_small.tile([64, 512], f32, tag="ps_small")
                        for p in range(NKT):
                            nc.tensor.matmul(
                                pv[:],
                                lhsT=wv_sb[:, slot, p, :],
                                rhs=xt_sb[:, p, ds(c * 512, 512)],
                                start=(p == 0), stop=(p == NKT - 1),
                            )
                        nc.vector.tensor_scalar_add(
                            vT[0:64, ds(c * 512, 512)], pv[:],
                            vbp_sb[:, slot:slot + 1],
                        )
                    vaug = vaugp.tile([128, 16, 65], bf16)
                    for t in range(16):
                        pt = ps_small.tile([128, 65], bf16, tag="ps_small")
                        nc.tensor.transpose(pt[:], vT[:, ts(t, 128)],
                                            ident_sb[0:65, 0:65])
                        nc.vector.tensor_copy(vaug[:, t, :], pt[:])
                else:
                    vaug = vdec_sb[:, :, slot - 5, :]

                if pending is not None:
                    emit_final(*pending)
                    pending = None
                if nphase < 2:
                    continue
                # ---------- scoresT -> exp -> mask, att half 0 interleaved --
                # exp (ACT) is the per-head pacer; att matmuls for sq half 0
                # run k-outer at lag 2 inside the scores loop so PE stays busy
                # (and warm) while ACT/DVE produce probs tiles.
                attu = attup.tile([66, S], f32, tag="attu")
                pa0 = ps_att.tile([65, 1024], f32, tag="ps_att")

                def att_mms(pa, kk, hoff):
                    for c in range(2):
                        nc.tensor.matmul(
                            pa[:, ds(c * 512, 512)],
                            lhsT=vaug[:, kk, :],
                            rhs=probs[kk][:, ds(hoff + c * 512, 512)],
                            start=(kk == 0), stop=(kk == 15),
                            skip_group_check=True,
                        )

                probs = []
                for k in range(16):
                    pk = probsp.tile([128, S], bf16, tag="probs")
                    probs.append(pk)
                    keep_t = keepp.tile([128, S], bf16, tag="keepp")
                    nc.sync.dma_start(out=keep_t[:], in_=keep_d[k, :, :])
                    for half in range(2):
                        psc = ps_scores.tile([128, 1024], f32, tag="ps_scores")
                        for cc in range(2):
                            nc.tensor.matmul(
                                psc[:, ds(cc * 512, 512)],
                                lhsT=kT[:, ts(k, 128)],
                                rhs=qT[:, ds(half * 1024 + cc * 512, 512)],
                                start=True, stop=True,
                            )
                        import concourse.mybir as mybir_  # noqa
                        nc.scalar.activation(
                            out=pk[:, ds(half * 1024, 1024)],
                            in_=psc[:],
                            func=mybir_.ActivationFunctionType.Exp,
                            scale=0.125,
                        )
                        nc.vector.tensor_mul(
                            pk[:, ds(half * 1024, 1024)],
                            pk[:, ds(half * 1024, 1024)],
                            keep_t[:, ds(half * 1024, 1024)],
                        )
                    if k >= 2:
                        att_mms(pa0, k - 2, 0)

                if nphase < 3:
                    continue
                # ---------- attention x V: finish half 0, then half 1 ----
                for kk in (14, 15):
                    att_mms(pa0, kk, 0)
                nc.vector.tensor_copy(attu[0:65, 0:1024], pa0[:])
                pa1 = ps_att.tile([65, 1024], f32, tag="ps_att")
                for kk in range(16):
                    att_mms(pa1, kk, 1024)
                nc.vector.tensor_copy(attu[0:65, 1024:2048], pa1[:])

                if nphase < 4:
                    continue
                # ---------- normalize: 1/sums broadcast ----------
                rb = recipbp.tile([64, S], f32, tag="recipb")
                nc.sync.dma_start(out=rb[0:1, :], in_=attu[64:65, :])
                nc.vector.reciprocal(rb[0:1, :], rb[0:1, :])
                nc.gpsimd.partition_broadcast(rb[:], rb[0:1, :])

                # attn_t pairs j=(2jj, 2jj+1) across partition halves so the
                # output linear contracts K=128 per pair-matmul. DVE lanes can
                # only write partitions 0:64, so odd-j blocks stage through
                # odd_t and DMA up to partitions 64:128.
                attn_t = attnp.tile([128, 8, 128], bf16, tag="attn")
                odd_t = oddp.tile([64, 8, 128], bf16, tag="odd")
                a_r = attu[0:64, :].rearrange("p (r jj e) -> p e jj r",
                                              jj=8, e=2)
                b_r = rb[:].rearrange("p (r jj e) -> p e jj r", jj=8, e=2)
                nc.vector.tensor_mul(attn_t[0:64, :, :],
                                     a_r[:, 0, :, :], b_r[:, 0, :, :])
                nc.vector.tensor_mul(odd_t[:], a_r[:, 1, :, :],
                                     b_r[:, 1, :, :])
                nc.sync.dma_start(out=attn_t[64:128, :, :], in_=odd_t[:])

                if nphase < 5:
                    continue
                pending = (slot, attn_t)
              if pending is not None:
                emit_final(*pending)
                pending = None

    nc.compile()
    return nc


def _prep_core_inputs(b, hh, dec_input, enc_input, keep16, W_qk_w, W_qk_b,
                      lin_in, lin_b16):
    import ml_dtypes
    bf16 = ml_dtypes.bfloat16
    heads = _heads_for(hh)
    enc_b = enc_input[b]
    dec_b = dec_input[b]
    encT = np.ascontiguousarray(enc_b.T)  # [1024, 2048]

    xt = np.empty((9, 128, S), np.float32)
    xt[:NKT] = encT.reshape(NKT, 128, S)
    if hh == 0:
        xt[8] = encT[896:1024]
    else:
        # head 11 q,k dec cols 64:192 transposed
        xt[8] = np.ascontiguousarray(dec_b[:, 64:192].T)

    qkdecT = np.empty((4, 64, S), np.float32)
    for i, slot in enumerate((6, 7)):
        h = heads[slot]
        mc = h * 192 - 2 * D  # dec col offset of this head's q
        qkdecT[2 * i] = dec_b[:, mc:mc + 64].T
        qkdecT[2 * i + 1] = dec_b[:, mc + 64:mc + 128].T

    vdec = np.empty((128, 16, 3, 65), np.float32)
    vdec[:, :, :, 64] = 1.0
    for blk, slot in enumerate((5, 6, 7)):
        h = heads[slot]
        mcv = h * 192 + 128 - 2 * D
        vcols = dec_b[:, mcv:mcv + 64]  # [2048, 64]
        vdec[:, :, blk, 0:64] = vcols.reshape(16, 128, 64).transpose(1, 0, 2)

    wqk = np.zeros((128, 6, NKT, 128), np.float32)
    qkb = np.zeros((128, 6), np.float32)
    for slot in range(6):
        h = heads[slot]
        if hh == 1 and slot == 5:
            wqk[:, slot, NKT - 1, :] = np.eye(128, dtype=np.float32)
        else:
            for p in range(NKT):
                wqk[:, slot, p, :] = W_qk_w[h * 192:h * 192 + 128,
                                            p * 128:(p + 1) * 128].T
            qkb[:, slot] = W_qk_b[h * 192:h * 192 + 128]

    wv = np.empty((128, 5, NKT, HD), np.float32)
    vbp = np.empty((64, 5), np.float32)
    for slot in range(5):
        h = heads[slot]
        for p in range(NKT):
            wv[:, slot, p, :] = W_qk_w[h * 192 + 128:h * 192 + 192,
                                       p * 128:(p + 1) * 128].T
        vbp[:, slot] = W_qk_b[h * 192 + 128:h * 192 + 192]

    return {
        "xt": xt.astype(bf16),
        "qkdect": qkdecT.astype(bf16),
        "vdec": vdec.astype(bf16),
        "wqk": wqk.astype(bf16),
        "qkb": qkb,
        "wv": wv.astype(bf16),
        "vbp": vbp,
        "keep": keep16,
        "lin": lin_in,
        "linb": lin_b16,
    }


def make_in_maps(dec_input, enc_input, mask, W_qk_w, W_qk_b, lin_w, lin_b):
    import ml_dtypes
    bf16 = ml_dtypes.bfloat16
    dec_input = np.asarray(dec_input, np.float32)
    enc_input = np.asarray(enc_input, np.float32)
    W_qk_w = np.asarray(W_qk_w, np.float32)
    W_qk_b = np.asarray(W_qk_b, np.float32)
    lin_w = np.asarray(lin_w, np.float32)
    lin_b = np.asarray(lin_b, np.float32)
    mask = np.asarray(mask)

    keep16 = np.ascontiguousarray(
        (~mask).T.astype(np.float32)).reshape(16, 128, S).astype(bf16)
    linT = np.ascontiguousarray(lin_w.T)  # [1024 (j,d), 1024 (n)]
    lin_in = np.ascontiguousarray(
        linT.reshape(8, 128, D).transpose(1, 0, 2)).astype(bf16)
    lin_b16 = lin_b.reshape(1, D).astype(bf16)

    in_maps = []
    for c in range(NCORES):
        b, hh = c // 2, c % 2
        in_maps.append(_prep_core_inputs(
            b, hh, dec_input, enc_input, keep16, W_qk_w, W_qk_b,
            lin_in, lin_b16))
    return in_maps


def get_nc():
    if "nc" not in _CACHE:
        _CACHE["nc"] = _build_nc()
    return _CACHE["nc"]


def gather_output(results):
    out = np.empty((B, S, D), np.float32)
    for c in range(NCORES):
        b, hh = c // 2, c % 2
        heads = _heads_for(hh)
        co = results[c]["out"]  # [8, 128, 1024]
        for slot, h in enumerate(heads):
            out[b, h * 128:(h + 1) * 128, :] = co[slot]
    return out


def kernel(dec_input, enc_input, mask, W_qk_w, W_qk_b, lin_w, lin_b):
    from concourse.bass_utils import run_bass_kernel_spmd

    nc = get_nc()
    in_maps = make_in_maps(dec_input, enc_input, mask, W_qk_w, W_qk_b,
                           lin_w, lin_b)
    res = run_bass_kernel_spmd(nc, in_maps, list(range(NCORES)))
    return gather_output(res.results)


# revision 31
# speedup vs baseline: 11.7653x; 1.0086x over previous
"""MultiHeadCrossAttention TRN2 kernel (8 NeuronCores, SPMD).

Sharding: core c -> (batch b = c // 2, head-half hh = c % 2).
Head-half hh owns heads [hh, hh+2, ..., hh+14] (interleaved so both halves
get the same mix of projection-sourced and dec-sourced heads).

Key structural fact: the reference's "raw reshape" out.reshape(B, S, D)
maps head h's attention output rows [0..2048) x [0..64) onto rows
[h*128, (h+1)*128) of the pre-linear activation matrix. So each head's
full pipeline (qkv -> attention -> output linear rows) is independent;
no cross-core communication is needed.

Per head slot (8 per core), all layouts chosen so PE contractions are on
the partition dim:
  qkT  [128=64q+64k, S]  = W_qk block @ enc^T   (or dec^T slices direct)
  v    [S, 64] (+ones)   = enc @ Wv^T           (or dec slices direct)
  scoresT[k,q] tiles     = kT.T @ qT            (PSUM, fp32)
  probsT = exp(0.125*scoresT) * keepT           (ACT exp + DVE mask mult)
  attT_unnorm [65, S]    = [v|1].T @ probsT     (row 64 = softmax sums)
  attT = attT_unnorm * (1/sums) broadcast       (DVE recip + DMA bcast)
  out rows [128, 1024]   = sum_j attT_j.T @ linT_j + lin_b
"""

import numpy as np

B, S, D, H, HD = 4, 2048, 1024, 16, 64
NCORES = 8
NKT = D // 128  # 8 K-tiles over the enc feature dim

_CACHE = {}


def _heads_for(hh):
    return list(range(hh, H, 2))


def _build_nc(nslots=8, nphase=5, nreps=1):
    import concourse.bass as bass
    import concourse.tile as tile
    from concourse import bacc, mybir

    f32 = mybir.dt.float32
    bf16 = mybir.dt.bfloat16
    ts, ds = bass.ts, bass.ds

    nc = bacc.Bacc("TRN2", target_bir_lowering=False, debug=False,
                   num_devices=NCORES)

    # DRAM I/O (per-core contents differ; program is identical)
    xt_d = nc.dram_tensor("xt", [9, 128, S], bf16, kind="ExternalInput")
    qkdecT_d = nc.dram_tensor("qkdect", [4, 64, S], bf16, kind="ExternalInput")
    vdec_d = nc.dram_tensor("vdec", [128, 16, 3, 65], bf16, kind="ExternalInput")
    wqk_d = nc.dram_tensor("wqk", [128, 6, NKT, 128], bf16, kind="ExternalInput")
    qkb_d = nc.dram_tensor("qkb", [128, 6], f32, kind="ExternalInput")
    wv_d = nc.dram_tensor("wv", [128, 5, NKT, HD], bf16, kind="ExternalInput")
    vbp_d = nc.dram_tensor("vbp", [64, 5], f32, kind="ExternalInput")
    keep_d = nc.dram_tensor("keep", [16, 128, S], bf16, kind="ExternalInput")
    lin_d = nc.dram_tensor("lin", [128, 8, D], bf16, kind="ExternalInput")
    linb_d = nc.dram_tensor("linb", [1, D], bf16, kind="ExternalInput")
    out_d = nc.dram_tensor("out", [8, 128, D], f32, kind="ExternalOutput")

    with tile.TileContext(nc) as tc:
        with (
            tc.tile_pool(name="consts", bufs=1) as consts,
            tc.tile_pool(name="keepp", bufs=3) as keepp,
            tc.tile_pool(name="qkt", bufs=1) as qktp,
            tc.tile_pool(name="kt0", bufs=1) as kt0p,
            tc.tile_pool(name="vaug", bufs=2) as vaugp,
            tc.tile_pool(name="vt", bufs=1) as vtp,
            tc.tile_pool(name="probs", bufs=16) as probsp,
            tc.tile_pool(name="attu", bufs=1) as attup,
            tc.tile_pool(name="recipb", bufs=1) as recipbp,
            tc.tile_pool(name="attn", bufs=1) as attnp,
            tc.tile_pool(name="odd", bufs=1) as oddp,
            tc.tile_pool(name="rstage", bufs=1) as rstagep,
            tc.tile_pool(name="ps_small", bufs=2, space="PSUM") as ps_small,
            tc.tile_pool(name="ps_scores", bufs=2, space="PSUM") as ps_scores,
            tc.tile_pool(name="ps_att", bufs=1, space="PSUM") as ps_att,
        ):
            # ---- resident constants ----
            xt_sb = consts.tile([128, 9, S], bf16)
            for i in range(9):
                nc.sync.dma_start(out=xt_sb[:, i, :], in_=xt_d[i, :, :])
            wqk_sb = consts.tile([128, 6, NKT, 128], bf16)
            nc.sync.dma_start(out=wqk_sb[:], in_=wqk_d[:, :, :, :])
            qkb_sb = consts.tile([128, 6], f32)
            nc.sync.dma_start(out=qkb_sb[:], in_=qkb_d[:, :])
            wv_sb = consts.tile([128, 5, NKT, HD], bf16)
            nc.sync.dma_start(out=wv_sb[:], in_=wv_d[:, :, :, :])
            qkdecT_sb = consts.tile([64, 4, S], bf16)
            for i in range(4):
                nc.sync.dma_start(out=qkdecT_sb[:, i, :], in_=qkdecT_d[i, :, :])
            vdec_sb = consts.tile([128, 16, 3, 65], bf16)
            nc.sync.dma_start(out=vdec_sb[:], in_=vdec_d[:, :, :, :])
            vbp_sb = consts.tile([64, 5], f32)
            nc.sync.dma_start(out=vbp_sb[:], in_=vbp_d[:, :])
            lin_sb = consts.tile([128, 8, D], bf16)
            nc.sync.dma_start(out=lin_sb[:], in_=lin_d[:, :, :])
            linb_sb = consts.tile([1, D], bf16)
            nc.sync.dma_start(out=linb_sb[:], in_=linb_d[:, :])
            ones_sb = consts.tile([1, 128], bf16)
            nc.vector.memset(ones_sb[:], 1.0)
            ident_sb = consts.tile([128, 128], bf16)
            from concourse.masks import make_identity
            make_identity(nc, ident_sb[:])

            def xt_idx(slot, p):
                # slot 5 pass 7 reads the per-core extra tile (index 8)
                return p if not (slot == 5 and p == NKT - 1) else 8

            def emit_final(slot, attn_t):
                # ---------- output linear (for the previous slot) ----------
                rst = rstagep.tile([128, D], f32, tag="rstage")
                for n in range(2):
                    pr = ps_small.tile([128, 512], f32, tag="ps_small")
                    nc.tensor.matmul(
                        pr[:],
                        lhsT=ones_sb[0:1, :],
                        rhs=linb_sb[0:1, ds(n * 512, 512)],
                        start=True, stop=False, skip_group_check=True,
                    )
                    for jj in range(8):
                        nc.tensor.matmul(
                            pr[:],
                            lhsT=attn_t[:, jj, :],
                            rhs=lin_sb[:, jj, ds(n * 512, 512)],
                            start=False, stop=(jj == 7),
                            skip_group_check=True,
                        )
                    nc.vector.tensor_copy(rst[:, ds(n * 512, 512)], pr[:])
                nc.sync.dma_start(out=out_d[slot, :, :], in_=rst[:])

            pending = None  # (slot, attn_t) awaiting its output linear
            for rep in range(nreps):
              prebuilt_vaug = None
              for slot in range(nslots):
                # ---------- projection ----------
                if slot < 6:
                    qkT = qktp.tile([128, S], bf16)
                    for c in range(4):
                        pq = ps_small.tile([128, 512], f32, tag="ps_small")
                        for p in range(NKT):
                            nc.tensor.matmul(
                                pq[:],
                                lhsT=wqk_sb[:, slot, p, :],
                                rhs=xt_sb[:, xt_idx(slot, p), ds(c * 512, 512)],
                                start=(p == 0),
                                stop=(p == NKT - 1),
                            )
                        nc.vector.tensor_scalar_add(
                            qkT[:, ds(c * 512, 512)], pq[:],
                            qkb_sb[:, slot:slot + 1],
                        )
                    # kT copy down to base partition 0 (matmul operands
                    # must share a base partition)
                    kT = kt0p.tile([64, S], bf16)
                    nc.sync.dma_start(out=kT[:], in_=qkT[64:128, :])
                    qT = qkT[0:64, :]
                    kT = kT[:]
                else:
                    qT = qkdecT_sb[:, 2 * (slot - 6), :]
                    kT = qkdecT_sb[:, 2 * (slot - 6) + 1, :]

                if slot < 5:
                    if prebuilt_vaug is not None:
                        vaug = prebuilt_vaug
                        prebuilt_vaug = None
                    else:
                        # vT [65, S]: rows 0:64 = Wv @ enc^T (+bias), row 64=1
                        vT = vtp.tile([65, S], bf16, tag="vt")
                        nc.vector.memset(vT[64:65, :], 1.0)
                        for c in range(4):
                            pv = ps_small.tile([64, 512], f32, tag="ps_small")
                            for p in range(NKT):
                                nc.tensor.matmul(
                                    pv[:],
                                    lhsT=wv_sb[:, slot, p, :],
                                    rhs=xt_sb[:, p, ds(c * 512, 512)],
                                    start=(p == 0), stop=(p == NKT - 1),
                                )
                            nc.vector.tensor_scalar_add(
                                vT[0:64, ds(c * 512, 512)], pv[:],
                                vbp_sb[:, slot:slot + 1],
                            )
                        vaug = vaugp.tile([128, 16, 65], bf16)
                        for t in range(16):
                            pt = ps_small.tile([128, 65], bf16,
                                               tag="ps_small")
                            nc.tensor.transpose(pt[:], vT[:, ts(t, 128)],
                                                ident_sb[0:65, 0:65])
                            nc.vector.tensor_copy(vaug[:, t, :], pt[:])
                else:
                    vaug = vdec_sb[:, :, slot - 5, :]

                if pending is not None:
                    emit_final(*pending)
                    pending = None
                if nphase < 2:
                    continue
                # ---------- scoresT -> exp -> mask, att half 0 interleaved --
                # exp (ACT) is the per-head pacer; att matmuls for sq half 0
                # run k-outer at lag 2 inside the scores loop so PE stays busy
                # (and warm) while ACT/DVE produce probs tiles.
                attu = attup.tile([66, S], f32, tag="attu")
                pa0 = ps_att.tile([65, 1024], f32, tag="ps_att")

                def att_mms(pa, kk, hoff):
                    for c in range(2):
                        nc.tensor.matmul(
                            pa[:, ds(c * 512, 512)],
                            lhsT=vaug[:, kk, :],
                            rhs=probs[kk][:, ds(hoff + c * 512, 512)],
                            start=(kk == 0), stop=(kk == 15),
                            skip_group_check=True,
                        )

                nxt = slot + 1
                do_next = nxt < min(nslots, 5)
                if do_next:
                    vTn = vtp.tile([65, S], bf16, tag="vt")
                    nc.vector.memset(vTn[64:65, :], 1.0)
                    vaugn = vaugp.tile([128, 16, 65], bf16)
                    pv_cur = None

                probs = []
                for k in range(16):
                    pk = probsp.tile([128, S], bf16, tag="probs")
                    probs.append(pk)
                    for half in range(2):
                        keep_t = keepp.tile([128, 1024], bf16, tag="keepp")
                        nc.sync.dma_start(
                            out=keep_t[:],
                            in_=keep_d[k, :, ds(half * 1024, 1024)])
                        psc = ps_scores.tile([128, 1024], f32, tag="ps_scores")
                        for cc in range(2):
                            nc.tensor.matmul(
                                psc[:, ds(cc * 512, 512)],
                                lhsT=kT[:, ts(k, 128)],
                                rhs=qT[:, ds(half * 1024 + cc * 512, 512)],
                                start=True, stop=True,
                            )
                        import concourse.mybir as mybir_  # noqa
                        nc.scalar.activation(
                            out=pk[:, ds(half * 1024, 1024)],
                            in_=psc[:],
                            func=mybir_.ActivationFunctionType.Exp,
                            scale=0.125,
                        )
                        nc.vector.tensor_mul(
                            pk[:, ds(half * 1024, 1024)],
                            pk[:, ds(half * 1024, 1024)],
                            keep_t[:],
                        )
                    if k >= 2:
                        att_mms(pa0, k - 2, 0)
                    # next slot's v-projection rides in the ACT-paced gaps
                    if do_next:
                        if k < 8:
                            c = k // 2
                            if k % 2 == 0:
                                pv_cur = ps_small.tile([64, 512], f32,
                                                       tag="ps_small")
                            for p in (range(4) if k % 2 == 0 else
                                      range(4, NKT)):
                                nc.tensor.matmul(
                                    pv_cur[:],
                                    lhsT=wv_sb[:, nxt, p, :],
                                    rhs=xt_sb[:, p, ds(c * 512, 512)],
                                    start=(p == 0), stop=(p == NKT - 1),
                                )
                            if k % 2 == 1:
                                nc.vector.tensor_scalar_add(
                                    vTn[0:64, ds(c * 512, 512)], pv_cur[:],
                                    vbp_sb[:, nxt:nxt + 1],
                                )
                        else:
                            for t in (2 * (k - 8), 2 * (k - 8) + 1):
                                pt = ps_small.tile([128, 65], bf16,
                                                   tag="ps_small")
                                nc.tensor.transpose(pt[:], vTn[:, ts(t, 128)],
                                                    ident_sb[0:65, 0:65])
                                nc.vector.tensor_copy(vaugn[:, t, :], pt[:])

                if do_next:
                    prebuilt_vaug = vaugn
                if nphase < 3:
                    continue
                # ---------- attention x V: finish half 0, then half 1 ----
                for kk in (14, 15):
                    att_mms(pa0, kk, 0)
                nc.vector.tensor_copy(attu[0:65, 0:1024], pa0[:])
                pa1 = ps_att.tile([65, 1024], f32, tag="ps_att")
                for kk in range(16):
                    att_mms(pa1, kk, 1024)
                nc.vector.tensor_copy(attu[0:65, 1024:2048], pa1[:])

                if nphase < 4:
                    continue
                # ---------- normalize: 1/sums broadcast ----------
                rb = recipbp.tile([64, S], f32, tag="recipb")
                nc.sync.dma_start(out=rb[0:1, :], in_=attu[64:65, :])
                nc.vector.reciprocal(rb[0:1, :], rb[0:1, :])
                nc.gpsimd.partition_broadcast(rb[:], rb[0:1, :])

                # attn_t pairs j=(2jj, 2jj+1) across partition halves so the
                # output linear contracts K=128 per pair-matmul. DVE lanes can
                # only write partitions 0:64, so odd-j blocks stage through
                # odd_t and DMA up to partitions 64:128.
                attn_t = attnp.tile([128, 8, 128], bf16, tag="attn")
                odd_t = oddp.tile([64, 8, 128], bf16, tag="odd")
                a_r = attu[0:64, :].rearrange("p (r jj e) -> p e jj r",
                                              jj=8, e=2)
                b_r = rb[:].rearrange("p (r jj e) -> p e jj r", jj=8, e=2)
                nc.vector.tensor_mul(attn_t[0:64, :, :],
                                     a_r[:, 0, :, :], b_r[:, 0, :, :])
                nc.vector.tensor_mul(odd_t[:], a_r[:, 1, :, :],
                                     b_r[:, 1, :, :])
                nc.sync.dma_start(out=attn_t[64:128, :, :], in_=odd_t[:])

                if nphase < 5:
                    continue
                pending = (slot, attn_t)
              if pending is not None:
                emit_final(*pending)
                pending = None

    nc.compile()
    return nc


def _prep_core_inputs(b, hh, dec_input, enc_input, keep16, W_qk_w, W_qk_b,
                      lin_in, lin_b16):
    import ml_dtypes
    bf16 = ml_dtypes.bfloat16
    heads = _heads_for(hh)
    enc_b = enc_input[b]
    dec_b = dec_input[b]
    encT = np.ascontiguousarray(enc_b.T)  # [1024, 2048]

    xt = np.empty((9, 128, S), np.float32)
    xt[:NKT] = encT.reshape(NKT, 128, S)
    if hh == 0:
        xt[8] = encT[896:1024]
    else:
        # head 11 q,k dec cols 64:192 transposed
        xt[8] = np.ascontiguousarray(dec_b[:, 64:192].T)

    qkdecT = np.empty((4, 64, S), np.float32)
    for i, slot in enumerate((6, 7)):
        h = heads[slot]
        mc = h * 192 - 2 * D  # dec col offset of this head's q
        qkdecT[2 * i] = dec_b[:, mc:mc + 64].T
        qkdecT[2 * i + 1] = dec_b[:, mc + 64:mc + 128].T

    vdec = np.empty((128, 16, 3, 65), np.float32)
    vdec[:, :, :, 64] = 1.0
    for blk, slot in enumerate((5, 6, 7)):
        h = heads[slot]
        mcv = h * 192 + 128 - 2 * D
        vcols = dec_b[:, mcv:mcv + 64]  # [2048, 64]
        vdec[:, :, blk, 0:64] = vcols.reshape(16, 128, 64).transpose(1, 0, 2)

    wqk = np.zeros((128, 6, NKT, 128), np.float32)
    qkb = np.zeros((128, 6), np.float32)
    for slot in range(6):
        h = heads[slot]
        if hh == 1 and slot == 5:
            wqk[:, slot, NKT - 1, :] = np.eye(128, dtype=np.float32)
        else:
            for p in range(NKT):
                wqk[:, slot, p, :] = W_qk_w[h * 192:h * 192 + 128,
                                            p * 128:(p + 1) * 128].T
            qkb[:, slot] = W_qk_b[h * 192:h * 192 + 128]

    wv = np.empty((128, 5, NKT, HD), np.float32)
    vbp = np.empty((64, 5), np.float32)
    for slot in range(5):
        h = heads[slot]
        for p in range(NKT):
            wv[:, slot, p, :] = W_qk_w[h * 192 + 128:h * 192 + 192,
                                       p * 128:(p + 1) * 128].T
        vbp[:, slot] = W_qk_b[h * 192 + 128:h * 192 + 192]

    return {
        "xt": xt.astype(bf16),
        "qkdect": qkdecT.astype(bf16),
        "vdec": vdec.astype(bf16),
        "wqk": wqk.astype(bf16),
        "qkb": qkb,
        "wv": wv.astype(bf16),
        "vbp": vbp,
        "keep": keep16,
        "lin": lin_in,
        "linb": lin_b16,
    }


def make_in_maps(dec_input, enc_input, mask, W_qk_w, W_qk_b, lin_w, lin_b):
    import ml_dtypes
    bf16 = ml_dtypes.bfloat16
    dec_input = np.asarray(dec_input, np.float32)
    enc_input = np.asarray(enc_input, np.float32)
    W_qk_w = np.asarray(W_qk_w, np.float32)
    W_qk_b = np.asarray(W_qk_b, np.float32)
    lin_w = np.asarray(lin_w, np.float32)
    lin_b = np.asarray(lin_b, np.float32)
    mask = np.asarray(mask)

    keep16 = np.ascontiguousarray(
        (~mask).T.astype(np.float32)).reshape(16, 128, S).astype(bf16)
    linT = np.ascontiguousarray(lin_w.T)  # [1024 (j,d), 1024 (n)]
    lin_in = np.ascontiguousarray(
        linT.reshape(8, 128, D).transpose(1, 0, 2)).astype(bf16)
    lin_b16 = lin_b.reshape(1, D).astype(bf16)

    in_maps = []
    for c in range(NCORES):
        b, hh = c // 2, c % 2
        in_maps.append(_prep_core_inputs(
            b, hh, dec_input, enc_input, keep16, W_qk_w, W_qk_b,
            lin_in, lin_b16))
    return in_maps


def get_nc():
    if "nc" not in _CACHE:
        _CACHE["nc"] = _build_nc()
    return _CACHE["nc"]


def gather_output(results):
    out = np.empty((B, S, D), np.float32)
    for c in range(NCORES):
        b, hh = c // 2, c % 2
        heads = _heads_for(hh)
        co = results[c]["out"]  # [8, 128, 1024]
        for slot, h in enumerate(heads):
            out[b, h * 128:(h + 1) * 128, :] = co[slot]
    return out


def kernel(dec_input, enc_input, mask, W_qk_w, W_qk_b, lin_w, lin_b):
    from concourse.bass_utils import run_bass_kernel_spmd

    nc = get_nc()
    in_maps = make_in_maps(dec_input, enc_input, mask, W_qk_w, W_qk_b,
                           lin_w, lin_b)
    res = run_bass_kernel_spmd(nc, in_maps, list(range(NCORES)))
    return gather_output(res.results)


# revision 34
# speedup vs baseline: 12.1207x; 1.0302x over previous
"""MultiHeadCrossAttention TRN2 kernel (8 NeuronCores, SPMD).

Sharding: core c -> (batch b = c // 2, head-half hh = c % 2).
Head-half hh owns heads [hh, hh+2, ..., hh+14] (interleaved so both halves
get the same mix of projection-sourced and dec-sourced heads).

Key structural fact: the reference's "raw reshape" out.reshape(B, S, D)
maps head h's attention output rows [0..2048) x [0..64) onto rows
[h*128, (h+1)*128) of the pre-linear activation matrix. So each head's
full pipeline (qkv -> attention -> output linear rows) is independent;
no cross-core communication is needed.

Per head slot (8 per core), all layouts chosen so PE contractions are on
the partition dim:
  qkT  [128=64q+64k, S]  = W_qk block @ enc^T   (or dec^T slices direct)
  v    [S, 64] (+ones)   = enc @ Wv^T           (or dec slices direct)
  scoresT[k,q] tiles     = kT.T @ qT            (PSUM, fp32)
  probsT = exp(0.125*scoresT) * keepT           (ACT exp + DVE mask mult)
  attT_unnorm [65, S]    = [v|1].T @ probsT     (row 64 = softmax sums)
  attT = attT_unnorm * (1/sums) broadcast       (DVE recip + DMA bcast)
  out rows [128, 1024]   = sum_j attT_j.T @ linT_j + lin_b
"""

import numpy as np

B, S, D, H, HD = 4, 2048, 1024, 16, 64
NCORES = 8
NKT = D // 128  # 8 K-tiles over the enc feature dim

_CACHE = {}


def _heads_for(hh):
    return list(range(hh, H, 2))


def _build_nc(nslots=8, nphase=5, nreps=1, fake_keep=False, lag=3):
    import concourse.bass as bass
    import concourse.tile as tile
    from concourse import bacc, mybir

    f32 = mybir.dt.float32
    bf16 = mybir.dt.bfloat16
    ts, ds = bass.ts, bass.ds

    nc = bacc.Bacc("TRN2", target_bir_lowering=False, debug=False,
                   num_devices=NCORES)

    # DRAM I/O (per-core contents differ; program is identical)
    xt_d = nc.dram_tensor("xt", [9, 128, S], bf16, kind="ExternalInput")
    qkdecT_d = nc.dram_tensor("qkdect", [4, 64, S], bf16, kind="ExternalInput")
    vdec_d = nc.dram_tensor("vdec", [128, 16, 3, 65], bf16, kind="ExternalInput")
    wqk_d = nc.dram_tensor("wqk", [128, 6, NKT, 128], bf16, kind="ExternalInput")
    qkb_d = nc.dram_tensor("qkb", [128, 6], f32, kind="ExternalInput")
    wv_d = nc.dram_tensor("wv", [128, 5, NKT, HD], bf16, kind="ExternalInput")
    vbp_d = nc.dram_tensor("vbp", [64, 5], f32, kind="ExternalInput")
    keep_d = nc.dram_tensor("keep", [16, 128, S], bf16, kind="ExternalInput")
    lin_d = nc.dram_tensor("lin", [128, 8, D], bf16, kind="ExternalInput")
    linb_d = nc.dram_tensor("linb", [1, D], bf16, kind="ExternalInput")
    out_d = nc.dram_tensor("out", [8, 128, D], f32, kind="ExternalOutput")

    with tile.TileContext(nc) as tc:
        with (
            tc.tile_pool(name="consts", bufs=1) as consts,
            tc.tile_pool(name="keepp", bufs=3) as keepp,
            tc.tile_pool(name="qkt", bufs=1) as qktp,
            tc.tile_pool(name="kt0", bufs=1) as kt0p,
            tc.tile_pool(name="vaug", bufs=2) as vaugp,
            tc.tile_pool(name="vt", bufs=1) as vtp,
            tc.tile_pool(name="probs", bufs=16) as probsp,
            tc.tile_pool(name="attu", bufs=1) as attup,
            tc.tile_pool(name="recipb", bufs=1) as recipbp,
            tc.tile_pool(name="attn", bufs=1) as attnp,
            tc.tile_pool(name="odd", bufs=1) as oddp,
            tc.tile_pool(name="rstage", bufs=1) as rstagep,
            tc.tile_pool(name="ps_small", bufs=2, space="PSUM") as ps_small,
            tc.tile_pool(name="ps_scores", bufs=2, space="PSUM") as ps_scores,
            tc.tile_pool(name="ps_att", bufs=1, space="PSUM") as ps_att,
        ):
            # ---- resident constants ----
            xt_sb = consts.tile([128, 9, S], bf16)
            for i in range(9):
                nc.sync.dma_start(out=xt_sb[:, i, :], in_=xt_d[i, :, :])
            wqk_sb = consts.tile([128, 6, NKT, 128], bf16)
            nc.sync.dma_start(out=wqk_sb[:], in_=wqk_d[:, :, :, :])
            qkb_sb = consts.tile([128, 6], f32)
            nc.sync.dma_start(out=qkb_sb[:], in_=qkb_d[:, :])
            wv_sb = consts.tile([128, 5, NKT, HD], bf16)
            nc.sync.dma_start(out=wv_sb[:], in_=wv_d[:, :, :, :])
            qkdecT_sb = consts.tile([64, 4, S], bf16)
            for i in range(4):
                nc.sync.dma_start(out=qkdecT_sb[:, i, :], in_=qkdecT_d[i, :, :])
            vdec_sb = consts.tile([128, 16, 3, 65], bf16)
            nc.sync.dma_start(out=vdec_sb[:], in_=vdec_d[:, :, :, :])
            vbp_sb = consts.tile([64, 5], f32)
            nc.sync.dma_start(out=vbp_sb[:], in_=vbp_d[:, :])
            lin_sb = consts.tile([128, 8, D], bf16)
            nc.sync.dma_start(out=lin_sb[:], in_=lin_d[:, :, :])
            linb_sb = consts.tile([1, D], bf16)
            nc.sync.dma_start(out=linb_sb[:], in_=linb_d[:, :])
            ones_sb = consts.tile([1, 128], bf16)
            nc.vector.memset(ones_sb[:], 1.0)
            ident_sb = consts.tile([128, 128], bf16)
            from concourse.masks import make_identity
            make_identity(nc, ident_sb[:])

            def xt_idx(slot, p):
                # slot 5 pass 7 reads the per-core extra tile (index 8)
                return p if not (slot == 5 and p == NKT - 1) else 8

            def emit_final(slot, attn_t):
                # ---------- output linear (for the previous slot) ----------
                rst = rstagep.tile([128, D], f32, tag="rstage")
                for n in range(2):
                    pr = ps_small.tile([128, 512], f32, tag="ps_small")
                    nc.tensor.matmul(
                        pr[:],
                        lhsT=ones_sb[0:1, :],
                        rhs=linb_sb[0:1, ds(n * 512, 512)],
                        start=True, stop=False, skip_group_check=True,
                    )
                    for jj in range(8):
                        nc.tensor.matmul(
                            pr[:],
                            lhsT=attn_t[:, jj, :],
                            rhs=lin_sb[:, jj, ds(n * 512, 512)],
                            start=False, stop=(jj == 7),
                            skip_group_check=True,
                        )
                    nc.vector.tensor_copy(rst[:, ds(n * 512, 512)], pr[:])
                nc.sync.dma_start(out=out_d[slot, :, :], in_=rst[:])

            pending = None  # (slot, attn_t) awaiting its output linear
            for rep in range(nreps):
              prebuilt_vaug = None
              for slot in range(nslots):
                # ---------- projection ----------
                if slot < 6:
                    qkT = qktp.tile([128, S], bf16)
                    for c in range(4):
                        pq = ps_small.tile([128, 512], f32, tag="ps_small")
                        for p in range(NKT):
                            nc.tensor.matmul(
                                pq[:],
                                lhsT=wqk_sb[:, slot, p, :],
                                rhs=xt_sb[:, xt_idx(slot, p), ds(c * 512, 512)],
                                start=(p == 0),
                                stop=(p == NKT - 1),
                            )
                        nc.vector.tensor_scalar_add(
                            qkT[:, ds(c * 512, 512)], pq[:],
                            qkb_sb[:, slot:slot + 1],
                        )
                    # kT copy down to base partition 0 (matmul operands
                    # must share a base partition)
                    kT = kt0p.tile([64, S], bf16)
                    nc.sync.dma_start(out=kT[:], in_=qkT[64:128, :])
                    qT = qkT[0:64, :]
                    kT = kT[:]
                else:
                    qT = qkdecT_sb[:, 2 * (slot - 6), :]
                    kT = qkdecT_sb[:, 2 * (slot - 6) + 1, :]

                if slot < 5:
                    if prebuilt_vaug is not None:
                        vaug = prebuilt_vaug
                        prebuilt_vaug = None
                    else:
                        # vT [65, S]: rows 0:64 = Wv @ enc^T (+bias), row 64=1
                        vT = vtp.tile([65, S], bf16, tag="vt")
                        nc.vector.memset(vT[64:65, :], 1.0)
                        for c in range(4):
                            pv = ps_small.tile([64, 512], f32, tag="ps_small")
                            for p in range(NKT):
                                nc.tensor.matmul(
                                    pv[:],
                                    lhsT=wv_sb[:, slot, p, :],
                                    rhs=xt_sb[:, p, ds(c * 512, 512)],
                                    start=(p == 0), stop=(p == NKT - 1),
                                )
                            nc.vector.tensor_scalar_add(
                                vT[0:64, ds(c * 512, 512)], pv[:],
                                vbp_sb[:, slot:slot + 1],
                            )
                        vaug = vaugp.tile([128, 16, 65], bf16)
                        for t in range(16):
                            pt = ps_small.tile([128, 65], bf16,
                                               tag="ps_small")
                            nc.tensor.transpose(pt[:], vT[:, ts(t, 128)],
                                                ident_sb[0:65, 0:65])
                            nc.vector.tensor_copy(vaug[:, t, :], pt[:])
                else:
                    vaug = vdec_sb[:, :, slot - 5, :]

                if pending is not None:
                    emit_final(*pending)
                    pending = None
                if nphase < 2:
                    continue
                # ---------- scoresT -> exp -> mask, att half 0 interleaved --
                # exp (ACT) is the per-head pacer; att matmuls for sq half 0
                # run k-outer at lag 2 inside the scores loop so PE stays busy
                # (and warm) while ACT/DVE produce probs tiles.
                attu = attup.tile([66, S], f32, tag="attu")
                pa0 = ps_att.tile([65, 1024], f32, tag="ps_att")

                def att_mms(pa, kk, hoff):
                    for c in range(2):
                        nc.tensor.matmul(
                            pa[:, ds(c * 512, 512)],
                            lhsT=vaug[:, kk, :],
                            rhs=probs[kk][:, ds(hoff + c * 512, 512)],
                            start=(kk == 0), stop=(kk == 15),
                            skip_group_check=True,
                        )

                nxt = slot + 1
                do_next = nxt < min(nslots, 5)
                if do_next:
                    vTn = vtp.tile([65, S], bf16, tag="vt")
                    nc.vector.memset(vTn[64:65, :], 1.0)
                    vaugn = vaugp.tile([128, 16, 65], bf16)
                    pv_cur = None

                probs = []
                for k in range(16):
                    pk = probsp.tile([128, S], bf16, tag="probs")
                    probs.append(pk)
                    for half in range(2):
                        keep_t = keepp.tile([128, 1024], bf16, tag="keepp")
                        if fake_keep:
                            nc.vector.memset(keep_t[:], 1.0)
                        else:
                            nc.sync.dma_start(
                                out=keep_t[:],
                                in_=keep_d[k, :, ds(half * 1024, 1024)])
                        psc = ps_scores.tile([128, 1024], f32, tag="ps_scores")
                        for cc in range(2):
                            nc.tensor.matmul(
                                psc[:, ds(cc * 512, 512)],
                                lhsT=kT[:, ts(k, 128)],
                                rhs=qT[:, ds(half * 1024 + cc * 512, 512)],
                                start=True, stop=True,
                            )
                        import concourse.mybir as mybir_  # noqa
                        nc.scalar.activation(
                            out=pk[:, ds(half * 1024, 1024)],
                            in_=psc[:],
                            func=mybir_.ActivationFunctionType.Exp,
                            scale=0.125,
                        )
                        nc.vector.tensor_mul(
                            pk[:, ds(half * 1024, 1024)],
                            pk[:, ds(half * 1024, 1024)],
                            keep_t[:],
                        )
                    if k >= lag:
                        att_mms(pa0, k - lag, 0)
                    # next slot's v-projection rides in the ACT-paced gaps
                    if do_next:
                        if k < 8:
                            c = k // 2
                            if k % 2 == 0:
                                pv_cur = ps_small.tile([64, 512], f32,
                                                       tag="ps_small")
                            for p in (range(4) if k % 2 == 0 else
                                      range(4, NKT)):
                                nc.tensor.matmul(
                                    pv_cur[:],
                                    lhsT=wv_sb[:, nxt, p, :],
                                    rhs=xt_sb[:, p, ds(c * 512, 512)],
                                    start=(p == 0), stop=(p == NKT - 1),
                                )
                            if k % 2 == 1:
                                nc.vector.tensor_scalar_add(
                                    vTn[0:64, ds(c * 512, 512)], pv_cur[:],
                                    vbp_sb[:, nxt:nxt + 1],
                                )
                        else:
                            for t in (2 * (k - 8), 2 * (k - 8) + 1):
                                pt = ps_small.tile([128, 65], bf16,
                                                   tag="ps_small")
                                nc.tensor.transpose(pt[:], vTn[:, ts(t, 128)],
                                                    ident_sb[0:65, 0:65])
                                nc.vector.tensor_copy(vaugn[:, t, :], pt[:])

                if do_next:
                    prebuilt_vaug = vaugn
                if nphase < 3:
                    continue
                # ---------- attention x V: finish half 0, then half 1 ----
                for kk in range(16 - lag, 16):
                    att_mms(pa0, kk, 0)
                nc.vector.tensor_copy(attu[0:65, 0:1024], pa0[:])
                pa1 = ps_att.tile([65, 1024], f32, tag="ps_att")
                for kk in range(16):
                    att_mms(pa1, kk, 1024)
                nc.vector.tensor_copy(attu[0:65, 1024:2048], pa1[:])

                if nphase < 4:
                    continue
                # ---------- normalize: 1/sums broadcast ----------
                rb = recipbp.tile([64, S], f32, tag="recipb")
                nc.sync.dma_start(out=rb[0:1, :], in_=attu[64:65, :])
                nc.vector.reciprocal(rb[0:1, :], rb[0:1, :])
                nc.gpsimd.partition_broadcast(rb[:], rb[0:1, :])

                # attn_t pairs j=(2jj, 2jj+1) across partition halves so the
                # output linear contracts K=128 per pair-matmul. DVE lanes can
                # only write partitions 0:64, so odd-j blocks stage through
                # odd_t and DMA up to partitions 64:128.
                attn_t = attnp.tile([128, 8, 128], bf16, tag="attn")
                odd_t = oddp.tile([64, 8, 128], bf16, tag="odd")
                a_r = attu[0:64, :].rearrange("p (r jj e) -> p e jj r",
                                              jj=8, e=2)
                b_r = rb[:].rearrange("p (r jj e) -> p e jj r", jj=8, e=2)
                nc.vector.tensor_mul(attn_t[0:64, :, :],
                                     a_r[:, 0, :, :], b_r[:, 0, :, :])
                nc.vector.tensor_mul(odd_t[:], a_r[:, 1, :, :],
                                     b_r[:, 1, :, :])
                nc.sync.dma_start(out=attn_t[64:128, :, :], in_=odd_t[:])

                if nphase < 5:
                    continue
                pending = (slot, attn_t)
              if pending is not None:
                emit_final(*pending)
                pending = None

    nc.compile()
    return nc


def _prep_core_inputs(b, hh, dec_input, enc_input, keep16, W_qk_w, W_qk_b,
                      lin_in, lin_b16):
    import ml_dtypes
    bf16 = ml_dtypes.bfloat16
    heads = _heads_for(hh)
    enc_b = enc_input[b]
    dec_b = dec_input[b]
    encT = np.ascontiguousarray(enc_b.T)  # [1024, 2048]

    xt = np.empty((9, 128, S), np.float32)
    xt[:NKT] = encT.reshape(NKT, 128, S)
    if hh == 0:
        xt[8] = encT[896:1024]
    else:
        # head 11 q,k dec cols 64:192 transposed
        xt[8] = np.ascontiguousarray(dec_b[:, 64:192].T)

    qkdecT = np.empty((4, 64, S), np.float32)
    for i, slot in enumerate((6, 7)):
        h = heads[slot]
        mc = h * 192 - 2 * D  # dec col offset of this head's q
        qkdecT[2 * i] = dec_b[:, mc:mc + 64].T
        qkdecT[2 * i + 1] = dec_b[:, mc + 64:mc + 128].T

    vdec = np.empty((128, 16, 3, 65), np.float32)
    vdec[:, :, :, 64] = 1.0
    for blk, slot in enumerate((5, 6, 7)):
        h = heads[slot]
        mcv = h * 192 + 128 - 2 * D
        vcols = dec_b[:, mcv:mcv + 64]  # [2048, 64]
        vdec[:, :, blk, 0:64] = vcols.reshape(16, 128, 64).transpose(1, 0, 2)

    wqk = np.zeros((128, 6, NKT, 128), np.float32)
    qkb = np.zeros((128, 6), np.float32)
    for slot in range(6):
        h = heads[slot]
        if hh == 1 and slot == 5:
            wqk[:, slot, NKT - 1, :] = np.eye(128, dtype=np.float32)
        else:
            for p in range(NKT):
                wqk[:, slot, p, :] = W_qk_w[h * 192:h * 192 + 128,
                                            p * 128:(p + 1) * 128].T
            qkb[:, slot] = W_qk_b[h * 192:h * 192 + 128]

    wv = np.empty((128, 5, NKT, HD), np.float32)
    vbp = np.empty((64, 5), np.float32)
    for slot in range(5):
        h = heads[slot]
        for p in range(NKT):
            wv[:, slot, p, :] = W_qk_w[h * 192 + 128:h * 192 + 192,
                                       p * 128:(p + 1) * 128].T
        vbp[:, slot] = W_qk_b[h * 192 + 128:h * 192 + 192]

    return {
        "xt": xt.astype(bf16),
        "qkdect": qkdecT.astype(bf16),
        "vdec": vdec.astype(bf16),
        "wqk": wqk.astype(bf16),
        "qkb": qkb,
        "wv": wv.astype(bf16),
        "vbp": vbp,
        "keep": keep16,
        "lin": lin_in,
        "linb": lin_b16,
    }


def make_in_maps(dec_input, enc_input, mask, W_qk_w, W_qk_b, lin_w, lin_b):
    import ml_dtypes
    bf16 = ml_dtypes.bfloat16
    dec_input = np.asarray(dec_input, np.float32)
    enc_input = np.asarray(enc_input, np.float32)
    W_qk_w = np.asarray(W_qk_w, np.float32)
    W_qk_b = np.asarray(W_qk_b, np.float32)
    lin_w = np.asarray(lin_w, np.float32)
    lin_b = np.asarray(lin_b, np.float32)
    mask = np.asarray(mask)

    keep16 = np.ascontiguousarray(
        (~mask).T.astype(np.float32)).reshape(16, 128, S).astype(bf16)
    linT = np.ascontiguousarray(lin_w.T)  # [1024 (j,d), 1024 (n)]
    lin_in = np.ascontiguousarray(
        linT.reshape(8, 128, D).transpose(1, 0, 2)).astype(bf16)
    lin_b16 = lin_b.reshape(1, D).astype(bf16)

    in_maps = []
    for c in range(NCORES):
        b, hh = c // 2, c % 2
        in_maps.append(_prep_core_inputs(
            b, hh, dec_input, enc_input, keep16, W_qk_w, W_qk_b,
            lin_in, lin_b16))
    return in_maps


def get_nc():
    if "nc" not in _CACHE:
        _CACHE["nc"] = _build_nc()
    return _CACHE["nc"]


def gather_output(results):
    out = np.empty((B, S, D), np.float32)
    for c in range(NCORES):
        b, hh = c // 2, c % 2
        heads = _heads_for(hh)
        co = results[c]["out"]  # [8, 128, 1024]
        for slot, h in enumerate(heads):
            out[b, h * 128:(h + 1) * 128, :] = co[slot]
    return out


def kernel(dec_input, enc_input, mask, W_qk_w, W_qk_b, lin_w, lin_b):
    from concourse.bass_utils import run_bass_kernel_spmd

    nc = get_nc()
    in_maps = make_in_maps(dec_input, enc_input, mask, W_qk_w, W_qk_b,
                           lin_w, lin_b)
    res = run_bass_kernel_spmd(nc, in_maps, list(range(NCORES)))
    return gather_output(res.results)
